# revision 1
# baseline (speedup 1.0000x reference)
"""Bidirectional 2nd-order IIR filter bank (64 channels) on 8 TRN2 NeuronCores.

Algorithm: block-parallel scan over the time axis.
  - T=24000 padded to 24576 = 192 blocks of L=128.
  - Phase A (per channel): zero-state particular solution of every block via a
    lower-triangular-Toeplitz matmul of the impulse response (PE, fp32r).
  - Block states are kept in REAL MODAL coordinates z = Vinv @ (y[n], y[n-1])
    (block propagation = scaled rotation, O(1) entries -> no 1/sin(theta)^2
    rounding amplification). Per-block state increments p~ are computed
    directly from x with a folded [128,2] matmul (batched across channels).
  - Phase B: second-level scan over the block states as lower-block-
    triangular matmuls, chunks of 64 blocks: chunk0+1 accumulate into the
    block-0..127 states (two K-halves), chunk2 uses the same half-0 weight
    columns plus a K=2 incoming-state matmul.
  - Phase C: homogeneous correction y += [g1 g2] @ z_in, a K=2 matmul
    accumulated onto the phase-A PSUM.
Sharding: 128 (direction x channel) independent filters; cores 0-3 forward
channels 0-63, cores 4-7 backward channels 0-63, 16 filters/core, B=8 local.
Output DRAM layout is [vc, n, col] (col = b*192 + block); the final
transpose to [b, c, t] happens on host in numpy.

Perf notes (v1 cost model): every DMA costs >=500ns on its dispatching
engine and a DMA's engine charge is bytes-per-partition, so DMA count and
thin transfers are minimized; DMAs are spread across SP / ACT / Pool
dispatchers. xrhs2 columns are chunk-interleaved (col = i*24 + chunk*8 + b)
so each vc's whole phase-B input is ONE plain contiguous DMA. SBUF pool-tile
APs in DMAs must be plain slices (no rearrange/unsqueeze views) or Tile
loses dependency tracking. Weight arrays are pre-arranged on host to the
exact SBUF layout so loads are plain 2D DMAs.
"""

import sys

import numpy as np

if "/opt/trn_rl_repo" not in sys.path:
    sys.path.insert(0, "/opt/trn_rl_repo")

T = 24000
B = 8
C = 64
L = 128
NBLK = 192
TPAD = NBLK * L  # 24576
NCOL = B * NBLK  # 1536
NVC = 16  # filters per core
NSL = 3  # 512-wide column slices
SLW = 512

_PROGRAM_CACHE = {}


# --------------------------------------------------------------------------
# host-side table construction (float64 -> float32, device SBUF layouts)
# --------------------------------------------------------------------------

def _tables_for_channels(a, b, chans):
    """Per-core weight tables for 16 channels. a,b: [64,3] float64."""
    NV = len(chans)
    wt = np.zeros((NV, 128, 128), np.float64)
    wb = np.zeros((NV, 2, 2, 128, 128), np.float64)
    hc = np.zeros((2, NV, 2, 64), np.float64)
    wp = np.zeros((128, 2 * NV), np.float64)
    g = np.zeros((2, NV * 128), np.float64)

    for vi, ch in enumerate(chans):
        a1, a2, b0 = a[ch, 1], a[ch, 2], b[ch, 0]
        r = np.sqrt(a2)
        costh = -a1 / (2.0 * r)
        sinth = np.sqrt(max(0.0, 1.0 - costh * costh))
        th = np.arctan2(sinth, costh)

        # impulse response h[m] = b0 * phi(m), phi: homogeneous w/ phi(0)=1
        h = np.zeros(130)
        h[0] = b0
        h[1] = -a1 * b0
        for m in range(2, 130):
            h[m] = -a1 * h[m - 1] - a2 * h[m - 2]

        # Toeplitz lhsT: wt[k, m] = h[m-k] for m >= k
        k_i = np.arange(128)
        d = k_i[None, :] - k_i[:, None]  # [k, m] -> m - k
        wt[vi] = np.where(d >= 0, h[np.clip(d, 0, 129)], 0.0)

        # modal decomposition: A = V S Vinv,
        # V = [[r c, r s],[1,0]], S = r[[c, s],[-s, c]],
        # Vinv = [[0,1],[1/(r s), -c/s]]
        Vinv = np.array([[0.0, 1.0], [1.0 / (r * sinth), -costh / sinth]])

        # wp = W_T[:, [127,126]] @ Vinv.T  (p~ = wp.T @ x_block)
        wp[:, 2 * vi:2 * vi + 2] = wt[vi][:, [127, 126]] @ Vinv.T

        # g[c, n] = (vrow0 @ S^{n+1})[c] = r^{n+2} (cos((n+2)th), sin((n+2)th))
        ks = np.arange(1, 129)  # k = n+1
        rk = r ** (ks + 1.0)
        g[0, vi * 128:(vi + 1) * 128] = rk * np.cos((ks + 1) * th)
        g[1, vi * 128:(vi + 1) * 128] = rk * np.sin((ks + 1) * th)

        # Mpow[q] = S^(128 q), q = 0..128: scaled rotations
        qs = np.arange(0, 129)
        rq = r ** (128.0 * qs)
        ang = 128.0 * qs * th
        M00 = rq * np.cos(ang)
        M01 = rq * np.sin(ang)
        # S^k = r^k [[cos, sin],[-sin, cos]] -> M10 = -M01, M11 = M00
        Mg = np.zeros((2, 2, 129))
        Mg[0, 0] = M00
        Mg[0, 1] = M01
        Mg[1, 0] = -M01
        Mg[1, 1] = M00

        # K-dim packing is c-major: kk = c*64 + i_local
        ii = np.arange(64)
        jj0 = np.arange(128)
        j1 = np.arange(64)
        for r2 in range(2):
            for half in range(2):
                dd = jj0[None, :] - (64 * half + ii[:, None])
                msk = dd >= 0
                dc = np.clip(dd, 0, 128)
                wb[vi, r2, half, 0:64, :] = np.where(msk, Mg[r2, 0, dc], 0.0)
                wb[vi, r2, half, 64:128, :] = np.where(msk, Mg[r2, 1, dc], 0.0)
            # chunk2 incoming-state weights: hc[c, vi, r2, j'] = (S^{128(j'+1)})[r2,c]
            hc[0, vi, r2, :] = Mg[r2, 0, j1 + 1]
            hc[1, vi, r2, :] = Mg[r2, 1, j1 + 1]

    # device SBUF layouts
    return {
        "wt": np.ascontiguousarray(
            wt.transpose(1, 0, 2).reshape(128, NV * 128)
        ).astype(np.float32),
        "wb": np.ascontiguousarray(
            wb.transpose(3, 0, 1, 2, 4).reshape(128, NV * 2 * 2 * 128)
        ).astype(np.float32),
        "hc": np.ascontiguousarray(hc.reshape(2, NV * 2 * 64)).astype(np.float32),
        "wp": wp.astype(np.float32),
        "g": g.astype(np.float32),
    }


# --------------------------------------------------------------------------
# device program
# --------------------------------------------------------------------------

def build_nc():
    """Build + compile the single-core Tile program (same on all 8 cores)."""
    import concourse.bass as bass
    import concourse.tile as tile
    from concourse import bacc, mybir

    f32 = mybir.dt.float32
    f32r = mybir.dt.float32r

    nc = bacc.Bacc("TRN2", target_bir_lowering=False, debug=False)

    xr_d = nc.dram_tensor("xrhs", [128, NCOL], f32r, kind="ExternalInput")
    xr2_d = nc.dram_tensor("xrhs2", [128, NCOL], f32r, kind="ExternalInput")
    wt_d = nc.dram_tensor("wt", [128, NVC * 128], f32r, kind="ExternalInput")
    wb_d = nc.dram_tensor("wb", [128, NVC * 2 * 2 * 128], f32, kind="ExternalInput")
    hc_d = nc.dram_tensor("hc", [2, NVC * 2 * 64], f32, kind="ExternalInput")
    wp_d = nc.dram_tensor("wp", [128, 2 * NVC], f32r, kind="ExternalInput")
    g_d = nc.dram_tensor("g", [2, NVC * 128], f32r, kind="ExternalInput")
    id_d = nc.dram_tensor("ident", [128, 128], f32, kind="ExternalInput")
    out_d = nc.dram_tensor("out", [NVC, 128, NCOL], f32, kind="ExternalOutput")

    with tile.TileContext(nc) as tc:
        with (
            tc.tile_pool(name="const", bufs=1) as const,
            tc.tile_pool(name="work", bufs=1) as work,
            tc.tile_pool(name="rp", bufs=6) as rp_pool,
            tc.tile_pool(name="sl", bufs=6) as sl_pool,
            tc.tile_pool(name="ss", bufs=5) as ss_pool,
            tc.tile_pool(name="tss", bufs=5) as tss_pool,
            tc.tile_pool(name="yout", bufs=6) as yout_pool,
            tc.tile_pool(name="sinp", bufs=6) as sin_pool,
            tc.tile_pool(name="bpsum", bufs=3, space="PSUM") as bpsum,
            tc.tile_pool(name="tpsum", bufs=2, space="PSUM") as tpsum,
            tc.tile_pool(name="opsum", bufs=3, space="PSUM") as opsum,
        ):
            # ---- constants into SBUF. Startup-latency aware: x + smalls on
            # SP (feed p~ and phase A early), wt/hc on ACT, wb split per-vc
            # on Pool so each vc's phase B unblocks as its weights land.
            xt2 = const.tile([128, NCOL], f32r)
            nc.sync.dma_start(xt2[:], xr2_d[:])
            wp_t = const.tile([128, 2 * NVC], f32r)
            nc.sync.dma_start(wp_t[:], wp_d[:])

            g_t = const.tile([2, NVC * 128], f32r)
            nc.sync.dma_start(g_t[:], g_d[:])
            id_t = const.tile([128, 128], f32)
            nc.sync.dma_start(id_t[:], id_d[:])
            xt = const.tile([128, NCOL], f32r)
            nc.scalar.dma_start(xt[:], xr_d[:])
            wt_t = const.tile([128, NVC * 128], f32r)
            nc.scalar.dma_start(wt_t[:], wt_d[:])
            hc_t = const.tile([2, NVC * 2 * 64], f32)
            nc.scalar.dma_start(hc_t[:], hc_d[:])
            wb_t = const.tile([128, NVC * 2 * 2 * 128], f32)
            WBW = 2 * 2 * 128  # per-vc wb columns
            for v in range(NVC):
                nc.gpsimd.dma_start(
                    wb_t[:, v * WBW:(v + 1) * WBW], wb_d[:, v * WBW:(v + 1) * WBW]
                )

            # ---- persistent work tiles
            pp_all = work.tile([32, NCOL], f32)

            # ---- p~ for all 16 vcs: one matmul per 512-col slice (xrhs2)
            for s in range(3):
                pt = opsum.tile([32, 512], f32, tag="o")
                nc.tensor.matmul(
                    pt[:],
                    wp_t[:],
                    xt2[:, s * 512:(s + 1) * 512],
                    start=True,
                    stop=True,
                )
                nc.vector.tensor_copy(pp_all[:, s * 512:(s + 1) * 512], pt[:])

            # ---- per-vc phases
            for v in range(NVC):
                ea = nc.sync if v % 2 == 0 else nc.scalar  # DMA dispatcher A
                # sins/outs spread so each dispatcher's total charge evens out
                e_sin = (nc.gpsimd, nc.sync, nc.gpsimd, nc.sync)[v % 4]
                e_out = (nc.sync, nc.gpsimd, nc.scalar, nc.sync)[v % 4]
                e_out2 = (nc.gpsimd, nc.scalar, nc.sync, nc.gpsimd)[v % 4]

                # whole phase-B input in ONE plain DMA: rp[kk = c*64+i,
                # chunk*8+b] <- pp rows (2v, 2v+1) (chunk-interleaved cols)
                rp = rp_pool.tile([128, 24], f32, tag="rp")
                ea.dma_start(rp[:], pp_all[2 * v:2 * v + 2, :])

                # blocks 0..127 states: S0[j, r*8+b], two K-halves
                s0 = bpsum.tile([128, 16], f32, tag="bp")
                for r2 in (0, 1):
                    base = ((v * 2 + r2) * 2) * 128
                    nc.tensor.matmul(
                        s0[:, r2 * 8:(r2 + 1) * 8],
                        wb_t[:, base:base + 128],
                        rp[:, 0:8],
                        start=True,
                        stop=False,
                    )
                    nc.tensor.matmul(
                        s0[:, r2 * 8:(r2 + 1) * 8],
                        wb_t[:, base + 128:base + 256],
                        rp[:, 8:16],
                        start=False,
                        stop=True,
                    )
                ss0 = ss_pool.tile([128, 16], f32, tag="ss0")
                nc.vector.tensor_copy(ss0[:], s0[:])

                # incoming state for chunk2 = s_127 comps (row 127 of S0)
                sl = sl_pool.tile([2, 8], f32, tag="sl")
                ea.dma_start(sl[:], ss0[127:128, 0:16])

                # blocks 128..191 states: in-chunk (= wb half-0 first 64 cols)
                # + K=2 incoming matmul
                s1 = bpsum.tile([64, 16], f32, tag="bp")
                for r2 in (0, 1):
                    base = ((v * 2 + r2) * 2) * 128
                    nc.tensor.matmul(
                        s1[:, r2 * 8:(r2 + 1) * 8],
                        wb_t[:, base:base + 64],
                        rp[:, 16:24],
                        start=True,
                        stop=False,
                    )
                    nc.tensor.matmul(
                        s1[:, r2 * 8:(r2 + 1) * 8],
                        hc_t[:, (v * 2 + r2) * 64:(v * 2 + r2 + 1) * 64],
                        sl[:],
                        start=False,
                        stop=True,
                    )
                ss1 = ss_pool.tile([64, 16], f32, tag="ss1")
                nc.vector.tensor_copy(ss1[:], s1[:])

                # transpose both chunks into one PSUM tile, then one copy:
                # tss col 0 = zeros, col 1+jj = s_jj
                tspu = tpsum.tile([16, 192], f32, tag="tp")
                nc.tensor.transpose(tspu[:, 0:128], ss0[:], id_t[:])
                nc.tensor.transpose(tspu[:, 128:192], ss1[:], id_t[0:64, 0:64])
                tss = tss_pool.tile([16, 193], f32r, tag="tss")
                nc.vector.tensor_copy(tss[:, 0:1], id_t[0:16, 100:101])  # zeros
                nc.vector.tensor_copy(tss[:, 1:193], tspu[:])

                # scatter into sin tile in ONE plain DMA (s_191 dropped):
                # sv[c, b*192+j] <- tss[c*8+b, j] (identical linear order)
                sv = sin_pool.tile([2, NCOL], f32r, tag="sin", name=f"sin{v}")
                e_sin.dma_start(sv[:, :], tss[:, 0:192])

                # phases A + C per 512-col slice, accumulate in PSUM, stage
                # the whole vc into one SBUF tile, single out-DMA
                yo = yout_pool.tile([128, NCOL], f32, tag="y")
                for s in range(NSL):
                    sli = slice(s * SLW, (s + 1) * SLW)
                    ps = opsum.tile([128, SLW], f32, tag="o")
                    nc.tensor.matmul(
                        ps[:],
                        wt_t[:, v * 128:(v + 1) * 128],
                        xt[:, sli],
                        start=True,
                        stop=False,
                    )
                    nc.tensor.matmul(
                        ps[:],
                        g_t[:, v * 128:(v + 1) * 128],
                        sv[:, sli],
                        start=False,
                        stop=True,
                    )
                    if s == 1:
                        nc.scalar.copy(yo[:, sli], ps[:])
                    else:
                        nc.vector.tensor_copy(yo[:, sli], ps[:])
                e_out.dma_start(out_d[v, :, 0:768], yo[:, 0:768])
                e_out2.dma_start(out_d[v, :, 768:1536], yo[:, 768:1536])

    nc.compile()
    return nc


def _get_program():
    if "nc" not in _PROGRAM_CACHE:
        _PROGRAM_CACHE["nc"] = build_nc()
    return _PROGRAM_CACHE["nc"]


# --------------------------------------------------------------------------
# host driver
# --------------------------------------------------------------------------

def make_in_maps(x, a_coeffs, b_coeffs):
    x = np.asarray(x, np.float32)
    a = np.asarray(a_coeffs, np.float64)
    b = np.asarray(b_coeffs, np.float64)
    xf = x[:, 0, :]

    def to_rhs(x2d):
        xpad = np.zeros((B, TPAD), np.float32)
        xpad[:, :T] = x2d
        return np.ascontiguousarray(
            xpad.reshape(B, NBLK, L).transpose(2, 0, 1).reshape(128, NCOL)
        )

    def to_rhs2(x2d):
        # chunk-interleaved block-major: X2[k, i*24 + c*8 + b]
        #   = xpad[b, (c*64+i)*128 + k]
        xpad = np.zeros((B, TPAD), np.float32)
        xpad[:, :T] = x2d
        return np.ascontiguousarray(
            xpad.reshape(B, 3, 64, L).transpose(3, 2, 1, 0).reshape(128, NCOL)
        )

    Xf = to_rhs(xf)
    Xb = to_rhs(xf[:, ::-1])
    X2f = to_rhs2(xf)
    X2b = to_rhs2(xf[:, ::-1])
    ident = np.eye(128, dtype=np.float32)

    in_maps = []
    for core in range(8):
        fwd = core < 4
        chans = list(range((core % 4) * NVC, (core % 4) * NVC + NVC))
        tabs = _tables_for_channels(a, b, chans)
        in_maps.append(
            {
                "xrhs": Xf if fwd else Xb,
                "xrhs2": X2f if fwd else X2b,
                "ident": ident,
                **tabs,
            }
        )
    return in_maps


def assemble_output(core_outs):
    y = np.zeros((B, 2 * C, T), np.float32)
    for core in range(8):
        o = np.asarray(core_outs[core])  # [16, 128, 1536]
        o = o.reshape(NVC, 128, B, NBLK).transpose(2, 0, 3, 1).reshape(B, NVC, TPAD)
        if core < 4:
            y[:, core * NVC:(core + 1) * NVC, :] = o[:, :, :T]
        else:
            y[:, C + (core - 4) * NVC:C + (core - 3) * NVC, :] = o[:, :, :T][:, :, ::-1]
    return y


def kernel(x, a_coeffs, b_coeffs, _trace=False):
    from concourse.bass_utils import run_bass_kernel_spmd

    nc = _get_program()
    in_maps = make_in_maps(x, a_coeffs, b_coeffs)
    res = run_bass_kernel_spmd(
        nc, in_maps, core_ids=list(range(8)), trace=_trace
    )
    y = assemble_output([r["out"] for r in res.results])
    if _trace:
        kernel.last_results = res
    return y



# revision 12
# speedup vs baseline: 1.2441x; 1.2441x over previous
"""Bidirectional 2nd-order IIR filter bank (64 channels) on 8 TRN2 NeuronCores.

Algorithm: block-parallel scan over the time axis (same math as the f32r
baseline, restructured for the v1 cost model where a DMA's engine charge is
free-dim bytes x 0.3855ns, min 500ns, on the dispatching engine).
  - T=24000 padded to 24576 = 192 blocks of L=128.
  - Phase A (per channel): zero-state particular solution of every block via a
    lower-triangular-Toeplitz matmul of the impulse response (PE, fp16).
  - Block states in REAL MODAL coordinates z = Vinv @ (y[n], y[n-1]); per-block
    increments p~ from a folded [128,2] matmul (xrhs2 layout).
  - Phase B: second-level scan as lower-block-triangular matmuls, 3 chunks of
    64 blocks; chunk2 adds a K=2 incoming-state matmul (sl).
  - States for ALL 16 filters are transposed per-batch (8 strided PE
    transposes) into ONE [32, NCOL] fp16 sv tile; phase C is then a K=32
    matmul with a host-built block-sparse g_all (rows 2v,2v+1 hold filter v's
    factors) accumulated onto phase A's PSUM. This avoids 16 expensive
    [2, NCOL] per-filter scatter DMAs entirely.
  - Everything on device is fp16 (PSUM accumulation f32) with global
    power-of-2 scaling: wt/beta, wp*alpha, g/(alpha*beta); the PSUM->SBUF
    output copy multiplies by beta. alpha=2^6, beta=2^-13 keeps every fp16
    stage in [1e-4, 200] (validated: rel_l2 ~5.4e-4 vs f64 reference).
Sharding: 128 (direction x channel) independent filters; cores 0-3 forward
channels 0-63, cores 4-7 backward channels 0-63, 16 filters/core, B=8 local.
Output DRAM layout is [vc, n, col] (col = b*192 + block) fp16; the final
transpose to [b, c, t] and f32 cast happen on host in numpy.

Perf notes (v1 cost model): DMA charge = free-bytes x 0.3855ns (x2 if the
min contiguous run < 512B), min 500ns, on the dispatching engine; DMA init
latency ~1.7us does not occupy the engine. Compute-op charge = free-size x
cycle_t + access bubble. Matmul charge = out free-size x 0.4167ns (fp16),
independent of K and partition count -- so thin-K matmuls and strided-AP
transposes are nearly free, and all small DMAs are batched (one sl DMA for
all 16 filters via the (r2, v, b) ss0 column layout).
"""

import sys

import numpy as np

if "/opt/trn_rl_repo" not in sys.path:
    sys.path.insert(0, "/opt/trn_rl_repo")

T = 24000
B = 8
C = 64
L = 128
NBLK = 192
TPAD = NBLK * L  # 24576
NCOL = B * NBLK  # 1536
NVC = 16  # filters per core
NSL = 3  # 512-wide column slices
SLW = 512

ALPHA = 2.0 ** 6
BETA = 2.0 ** -13

_PROGRAM_CACHE = {}


# --------------------------------------------------------------------------
# host-side table construction (float64 -> float16, device SBUF layouts)
# --------------------------------------------------------------------------

def _tables_for_channels(a, b, chans):
    """Per-core weight tables for 16 channels. a,b: [64,3] float64."""
    NV = len(chans)
    wt = np.zeros((NV, 128, 128), np.float64)
    wb = np.zeros((NV, 2, 2, 128, 128), np.float64)  # [v, r2, half, (c,i), j]
    hc = np.zeros((NV, 2, 2, 64), np.float64)  # [v, c, r2, j']
    wp = np.zeros((128, 2 * NV), np.float64)
    ga = np.zeros((64, NV * 128), np.float64)  # rows r2*32+v, col block v

    for vi, ch in enumerate(chans):
        a1, a2, b0 = a[ch, 1], a[ch, 2], b[ch, 0]
        r = np.sqrt(a2)
        costh = -a1 / (2.0 * r)
        sinth = np.sqrt(max(0.0, 1.0 - costh * costh))
        th = np.arctan2(sinth, costh)

        # impulse response h[m] = b0 * phi(m), phi: homogeneous w/ phi(0)=1
        h = np.zeros(130)
        h[0] = b0
        h[1] = -a1 * b0
        for m in range(2, 130):
            h[m] = -a1 * h[m - 1] - a2 * h[m - 2]

        # Toeplitz lhsT: wt[k, m] = h[m-k] for m >= k
        k_i = np.arange(128)
        d = k_i[None, :] - k_i[:, None]  # [k, m] -> m - k
        wt[vi] = np.where(d >= 0, h[np.clip(d, 0, 129)], 0.0)

        # modal decomposition: A = V S Vinv,
        # V = [[r c, r s],[1,0]], S = r[[c, s],[-s, c]],
        # Vinv = [[0,1],[1/(r s), -c/s]]
        Vinv = np.array([[0.0, 1.0], [1.0 / (r * sinth), -costh / sinth]])

        # wp = W_T[:, [127,126]] @ Vinv.T  (p~ = wp.T @ x_block)
        wp[:, 2 * vi:2 * vi + 2] = wt[vi][:, [127, 126]] @ Vinv.T

        # g[c, n] = r^{n+2} (cos((n+2)th), sin((n+2)th)); block-sparse rows
        ks = np.arange(1, 129)
        rk = r ** (ks + 1.0)
        ga[vi, vi * 128:(vi + 1) * 128] = rk * np.cos((ks + 1) * th)
        ga[32 + vi, vi * 128:(vi + 1) * 128] = rk * np.sin((ks + 1) * th)

        # Mpow[q] = S^(128 q): scaled rotations
        qs = np.arange(0, 129)
        rq = r ** (128.0 * qs)
        ang = 128.0 * qs * th
        Mg = np.zeros((2, 2, 129))
        Mg[0, 0] = rq * np.cos(ang)
        Mg[0, 1] = rq * np.sin(ang)
        Mg[1, 0] = -Mg[0, 1]
        Mg[1, 1] = Mg[0, 0]

        # K-dim packing is c-major: kk = c*64 + i (i = block within chunk)
        ii = np.arange(64)
        jj = np.arange(128)
        j1 = np.arange(64)
        for r2 in range(2):
            for half in range(2):
                dd = jj[None, :] - (64 * half + ii[:, None])
                msk = dd >= 0
                dc = np.clip(dd, 0, 128)
                wb[vi, r2, half, 0:64, :] = np.where(msk, Mg[r2, 0, dc], 0.0)
                wb[vi, r2, half, 64:128, :] = np.where(msk, Mg[r2, 1, dc], 0.0)
            # chunk2 incoming-state weights: hc[c, r2, j'] = (S^{128(j'+1)})[r2,c]
            hc[vi, 0, r2, :] = Mg[r2, 0, j1 + 1]
            hc[vi, 1, r2, :] = Mg[r2, 1, j1 + 1]

    # device SBUF layouts, fp16 with global scaling
    return {
        "wt": np.ascontiguousarray(
            (wt / BETA).transpose(1, 0, 2).reshape(128, NV * 128)
        ).astype(np.float16),
        "wb": np.ascontiguousarray(
            wb.transpose(3, 0, 1, 2, 4).reshape(128, NV * 2 * 2 * 128)
        ).astype(np.float16),
        "hc": np.ascontiguousarray(
            hc.transpose(1, 0, 2, 3).reshape(2, NV * 2 * 64)
        ).astype(np.float16),
        "wp": (wp * ALPHA).astype(np.float16),
        "gall": (ga / (ALPHA * BETA)).astype(np.float16),
    }


# --------------------------------------------------------------------------
# device program
# --------------------------------------------------------------------------

def build_nc():
    """Build + compile the single-core Tile program (same on all 8 cores)."""
    import concourse.bass as bass
    import concourse.tile as tile
    from concourse import bacc, mybir

    f32 = mybir.dt.float32
    f16 = mybir.dt.float16

    nc = bacc.Bacc("TRN2", target_bir_lowering=False, debug=False)

    xr_d = nc.dram_tensor("xrhs", [128, NCOL], f16, kind="ExternalInput")
    xr2_d = nc.dram_tensor("xrhs2", [128, NCOL], f16, kind="ExternalInput")
    wt_d = nc.dram_tensor("wt", [128, NVC * 128], f16, kind="ExternalInput")
    wb_d = nc.dram_tensor("wb", [128, NVC * 512], f16, kind="ExternalInput")
    hc_d = nc.dram_tensor("hc", [2, NVC * 128], f16, kind="ExternalInput")
    wp_d = nc.dram_tensor("wp", [128, 2 * NVC], f16, kind="ExternalInput")
    ga_d = nc.dram_tensor("gall", [64, NVC * 128], f16, kind="ExternalInput")
    id_d = nc.dram_tensor("ident", [128, 128], f16, kind="ExternalInput")
    out_d = nc.dram_tensor("out", [NVC, 128, NCOL], f16, kind="ExternalOutput")

    with tile.TileContext(nc) as tc:
        with (
            tc.tile_pool(name="const", bufs=1) as const,
            tc.tile_pool(name="work", bufs=1) as work,
            tc.tile_pool(name="rp", bufs=16) as rp_pool,
            tc.tile_pool(name="yout", bufs=4) as yout_pool,
            tc.tile_pool(name="bpsum", bufs=2, space="PSUM") as bpsum,
            tc.tile_pool(name="opsum", bufs=3, space="PSUM") as opsum,
        ):
            # ---- constants into SBUF. Startup-latency aware: xrhs2/wp first
            # (feed p~), then the phase-B tables, then phase-A tables.
            xt2 = const.tile([128, NCOL], f16)
            nc.sync.dma_start(xt2[:], xr2_d[:])
            wp_t = const.tile([128, 2 * NVC], f16)
            nc.sync.dma_start(wp_t[:], wp_d[:])
            wb_t = const.tile([128, NVC * 512], f16)
            for q in range(4):
                nc.gpsimd.dma_start(
                    wb_t[:, q * 2048:(q + 1) * 2048], wb_d[:, q * 2048:(q + 1) * 2048]
                )
            id_t = const.tile([128, 128], f16)
            nc.gpsimd.dma_start(id_t[:], id_d[:])
            hc_t = const.tile([2, NVC * 128], f16)
            nc.gpsimd.dma_start(hc_t[:], hc_d[:])
            xt = const.tile([128, NCOL], f16)
            nc.sync.dma_start(xt[:], xr_d[:])
            wt_t = const.tile([128, NVC * 128], f16)
            nc.sync.dma_start(wt_t[:], wt_d[:])
            g_t = const.tile([64, NVC * 128], f16)
            nc.gpsimd.dma_start(g_t[:], ga_d[:])

            # ---- persistent work tiles
            pp_all = work.tile([32, NCOL], f16)
            ss0_all = work.tile([128, 256], f16)  # cols r2*128 + v*8 + b
            ss1_all = work.tile([64, 256], f16)
            sl_all = work.tile([2, 128], f16)  # [r2, v*8+b]
            sv_all = work.tile([64, NCOL], f16)  # rows r2*32+v, cols b*192+n
            # zero once, early: covers the per-batch zero-state columns AND
            # the padding rows (16:32, 48:64) the K=64 phase-C rhs reads
            nc.vector.memset(sv_all[:], 0.0)

            # ---- p~ for all 16 vcs: one matmul per 512-col slice (xrhs2)
            pp_copy_eng = (nc.vector, nc.scalar, nc.vector)
            for s in range(3):
                pt = opsum.tile([32, SLW], f32, tag="o")
                nc.tensor.matmul(
                    pt[:], wp_t[:], xt2[:, s * SLW:(s + 1) * SLW],
                    start=True, stop=True,
                )
                eng = pp_copy_eng[s]
                if eng is nc.scalar:
                    eng.copy(pp_all[:, s * SLW:(s + 1) * SLW], pt[:])
                else:
                    eng.tensor_copy(pp_all[:, s * SLW:(s + 1) * SLW], pt[:])

            # ---- phase B: rp loads + s0 matmuls + ss0 copies (all vcs)
            rp_eng = (nc.sync, nc.scalar, nc.sync, nc.sync, nc.scalar, nc.sync,
                      nc.sync, nc.scalar, nc.sync, nc.sync, nc.scalar, nc.sync,
                      nc.sync, nc.scalar, nc.sync, nc.scalar)
            s0_copy_eng = (nc.vector, nc.scalar)
            rps = []
            for v in range(NVC):
                rp = rp_pool.tile([128, 24], f16, tag="rp", name=f"rp{v}")
                rp_eng[v].dma_start(rp[:], pp_all[2 * v:2 * v + 2, :])
                rps.append(rp)

            for v in range(NVC):
                rp = rps[v]
                s0 = bpsum.tile([128, 16], f32, tag="bp")
                for r2 in (0, 1):
                    base = v * 512 + r2 * 256
                    cs = slice(r2 * 8, r2 * 8 + 8)
                    nc.tensor.matmul(
                        s0[:, cs], wb_t[:, base:base + 128],
                        rp[:, 0:8], start=True, stop=False,
                    )
                    nc.tensor.matmul(
                        s0[:, cs], wb_t[:, base + 128:base + 256],
                        rp[:, 8:16], start=False, stop=True,
                    )
                # ss0_all[:, r2*128 + v*8 + b] <- s0[:, r2*8 + b]
                dst = ss0_all[:].rearrange("p (r w) -> p r w", r=2, w=128)
                src = s0[:].rearrange("p (r b) -> p r b", r=2, b=8)
                eng = s0_copy_eng[v % 2]
                if eng is nc.scalar:
                    eng.copy(dst[:, :, v * 8:(v + 1) * 8], src)
                else:
                    eng.tensor_copy(dst[:, :, v * 8:(v + 1) * 8], src)

            # incoming states for chunk2: one DMA for all vcs
            nc.sync.dma_start(sl_all[:], ss0_all[127:128, :])

            # ---- s1 matmuls + ss1 copies
            for v in range(NVC):
                rp = rps[v]
                s1 = bpsum.tile([64, 16], f32, tag="bp")
                for r2 in (0, 1):
                    base = v * 512 + r2 * 256
                    hbase = v * 128 + r2 * 64
                    cs = slice(r2 * 8, r2 * 8 + 8)
                    nc.tensor.matmul(
                        s1[:, cs], wb_t[:, base:base + 64],
                        rp[:, 16:24], start=True, stop=False,
                    )
                    nc.tensor.matmul(
                        s1[:, cs], hc_t[:, hbase:hbase + 64],
                        sl_all[:, v * 8:(v + 1) * 8], start=False, stop=True,
                    )
                dst = ss1_all[:].rearrange("p (r w) -> p r w", r=2, w=128)
                src = s1[:].rearrange("p (r b) -> p r b", r=2, b=8)
                eng = s0_copy_eng[v % 2]
                if eng is nc.scalar:
                    eng.copy(dst[:, :, v * 8:(v + 1) * 8], src)
                else:
                    eng.tensor_copy(dst[:, :, v * 8:(v + 1) * 8], src)

            # ---- per-(batch, component) strided transposes: 16 vcs at once.
            # psv_r[r2] cols: A-blocks b*128 (s_0..s_127), B-blocks 1024+b*64
            psv_r = [bpsum.tile([16, 1536], f16, tag="bp", name=f"psv{r}")
                     for r in range(2)]
            for r2 in range(2):
                for b in range(8):
                    c0 = r2 * 128 + b
                    nc.tensor.transpose(
                        psv_r[r2][:, b * 128:(b + 1) * 128],
                        ss0_all[:, c0:c0 + 121:8], id_t[:],
                    )
                    nc.tensor.transpose(
                        psv_r[r2][:, 1024 + b * 64:1024 + (b + 1) * 64],
                        ss1_all[:, c0:c0 + 121:8], id_t[0:64, 0:64],
                    )
            # sv_all[r2*32+v, b*192+n] = s_{n-1}; col b*192 stays zero
            sv_copy_eng = (nc.vector, nc.scalar)
            for r2 in range(2):
                dst = sv_all[r2 * 32:r2 * 32 + 16, :].rearrange(
                    "p (b n) -> p b n", b=8, n=192
                )
                in1 = psv_r[r2][:, 0:1024].rearrange("p (b n) -> p b n", b=8, n=128)
                in2 = psv_r[r2][:, 1024:1536].rearrange(
                    "p (b n) -> p b n", b=8, n=64
                )[:, :, 0:63]
                eng = sv_copy_eng[r2]
                if eng is nc.scalar:
                    eng.copy(dst[:, :, 1:129], in1)
                    eng.copy(dst[:, :, 129:192], in2)
                else:
                    eng.tensor_copy(dst[:, :, 1:129], in1)
                    eng.tensor_copy(dst[:, :, 129:192], in2)

            # ---- phases A + C per 512-col slice; K=32 block-sparse g matmul
            # accumulates the state correction onto the phase-A PSUM.
            V, A = nc.vector, nc.scalar
            # 16-long rotation x3 = 48 yo copies: DVE ~30, ACT ~18
            yo_copy_eng = (V, A, V, V, A, V, A, V, V, A, V, A, V, V, A, V)
            e_out = (nc.sync, nc.gpsimd) * 8
            e_out2 = (nc.gpsimd, nc.sync) * 8
            for v in range(NVC):
                yo = yout_pool.tile([128, NCOL], f16, tag="y", name=f"yo{v}")
                for s in range(NSL):
                    sli = slice(s * SLW, (s + 1) * SLW)
                    ps = opsum.tile([128, SLW], f32, tag="o")
                    nc.tensor.matmul(
                        ps[:], g_t[:, v * 128:(v + 1) * 128], sv_all[:, sli],
                        start=True, stop=False,
                    )
                    nc.tensor.matmul(
                        ps[:], wt_t[:, v * 128:(v + 1) * 128], xt[:, sli],
                        start=False, stop=True,
                    )
                    eng = yo_copy_eng[(v * NSL + s) % 16]
                    if eng is nc.scalar:
                        eng.copy(yo[:, sli], ps[:])
                    else:
                        eng.tensor_copy(yo[:, sli], ps[:])
                e_out[v].dma_start(out_d[v, :, 0:768], yo[:, 0:768])
                e_out2[v].dma_start(out_d[v, :, 768:1536], yo[:, 768:1536])

    nc.compile()
    return nc


def _get_program():
    if "nc" not in _PROGRAM_CACHE:
        _PROGRAM_CACHE["nc"] = build_nc()
    return _PROGRAM_CACHE["nc"]


# --------------------------------------------------------------------------
# host driver
# --------------------------------------------------------------------------

def make_in_maps(x, a_coeffs, b_coeffs):
    x = np.asarray(x, np.float32)
    a = np.asarray(a_coeffs, np.float64)
    b = np.asarray(b_coeffs, np.float64)
    xf = x[:, 0, :]

    def to_rhs(x2d):
        xpad = np.zeros((B, TPAD), np.float32)
        xpad[:, :T] = x2d
        return np.ascontiguousarray(
            xpad.reshape(B, NBLK, L).transpose(2, 0, 1).reshape(128, NCOL)
        ).astype(np.float16)

    def to_rhs2(x2d):
        # chunk-interleaved block-major: X2[k, i*24 + c*8 + b]
        #   = xpad[b, (c*64+i)*128 + k]
        xpad = np.zeros((B, TPAD), np.float32)
        xpad[:, :T] = x2d
        return np.ascontiguousarray(
            xpad.reshape(B, 3, 64, L).transpose(3, 2, 1, 0).reshape(128, NCOL)
        ).astype(np.float16)

    Xf = to_rhs(xf)
    Xb = to_rhs(xf[:, ::-1])
    X2f = to_rhs2(xf)
    X2b = to_rhs2(xf[:, ::-1])
    ident = np.eye(128, dtype=np.float16)

    in_maps = []
    for core in range(8):
        fwd = core < 4
        chans = list(range((core % 4) * NVC, (core % 4) * NVC + NVC))
        tabs = _tables_for_channels(a, b, chans)
        in_maps.append(
            {
                "xrhs": Xf if fwd else Xb,
                "xrhs2": X2f if fwd else X2b,
                "ident": ident,
                **tabs,
            }
        )
    return in_maps


def assemble_output(core_outs):
    y = np.zeros((B, 2 * C, T), np.float32)
    for core in range(8):
        o = np.asarray(core_outs[core]).astype(np.float32) * BETA  # [16, 128, 1536]
        o = o.reshape(NVC, 128, B, NBLK).transpose(2, 0, 3, 1).reshape(B, NVC, TPAD)
        if core < 4:
            y[:, core * NVC:(core + 1) * NVC, :] = o[:, :, :T]
        else:
            y[:, C + (core - 4) * NVC:C + (core - 3) * NVC, :] = o[:, :, :T][:, :, ::-1]
    return y


def kernel(x, a_coeffs, b_coeffs, _trace=False):
    from concourse.bass_utils import run_bass_kernel_spmd

    nc = _get_program()
    in_maps = make_in_maps(x, a_coeffs, b_coeffs)
    res = run_bass_kernel_spmd(
        nc, in_maps, core_ids=list(range(8)), trace=_trace
    )
    y = assemble_output([r["out"] for r in res.results])
    if _trace:
        kernel.last_results = res
    return y


# revision 15
# speedup vs baseline: 1.3635x; 1.0960x over previous
"""Bidirectional 2nd-order IIR filter bank (64 channels) on 8 TRN2 NeuronCores.

Algorithm: block-parallel scan over the time axis (same math as the f32r
baseline, restructured for the v1 cost model where a DMA's engine charge is
free-dim bytes x 0.3855ns, min 500ns, on the dispatching engine).
  - T=24000 padded to 24576 = 192 blocks of L=128.
  - Phase A (per channel): zero-state particular solution of every block via a
    lower-triangular-Toeplitz matmul of the impulse response (PE, fp16).
  - Block states in REAL MODAL coordinates z = Vinv @ (y[n], y[n-1]); per-block
    increments p~ from a folded [128,2] matmul (xrhs2 layout).
  - Phase B: second-level scan as lower-block-triangular matmuls, 3 chunks of
    64 blocks; chunk2 adds a K=2 incoming-state matmul (sl).
  - States for ALL 16 filters are transposed per-batch (8 strided PE
    transposes) into ONE [32, NCOL] fp16 sv tile; phase C is then a K=32
    matmul with a host-built block-sparse g_all (rows 2v,2v+1 hold filter v's
    factors) accumulated onto phase A's PSUM. This avoids 16 expensive
    [2, NCOL] per-filter scatter DMAs entirely.
  - Everything on device is fp16 (PSUM accumulation f32) with global
    power-of-2 scaling: wt/beta, wp*alpha, g/(alpha*beta); the PSUM->SBUF
    output copy multiplies by beta. alpha=2^6, beta=2^-13 keeps every fp16
    stage in [1e-4, 200] (validated: rel_l2 ~5.4e-4 vs f64 reference).
Sharding: 128 (direction x channel) independent filters; cores 0-3 forward
channels 0-63, cores 4-7 backward channels 0-63, 16 filters/core, B=8 local.
Output DRAM layout is [vc, n, col] (col = b*192 + block) fp16; the final
transpose to [b, c, t] and f32 cast happen on host in numpy.

Perf notes (v1 cost model): DMA charge = free-bytes x 0.3855ns (x2 if the
min contiguous run < 512B), min 500ns, on the dispatching engine; DMA init
latency ~1.7us does not occupy the engine. Compute-op charge = free-size x
cycle_t + access bubble. Matmul charge = out free-size x 0.4167ns (fp16),
independent of K and partition count -- so thin-K matmuls and strided-AP
transposes are nearly free, and all small DMAs are batched (one sl DMA for
all 16 filters via the (r2, v, b) ss0 column layout).
"""

import sys

import numpy as np

if "/opt/trn_rl_repo" not in sys.path:
    sys.path.insert(0, "/opt/trn_rl_repo")

T = 24000
B = 8
C = 64
L = 128
NBLK = 192
TPAD = NBLK * L  # 24576
NCOL = B * NBLK  # 1536
NVC = 16  # filters per core
NSL = 3  # 512-wide column slices
SLW = 512

ALPHA = 2.0 ** 6
BETA = 2.0 ** -13

_PROGRAM_CACHE = {}


# --------------------------------------------------------------------------
# host-side table construction (float64 -> float16, device SBUF layouts)
# --------------------------------------------------------------------------

def _tables_for_channels(a, b, chans):
    """Per-core weight tables for 16 channels. a,b: [64,3] float64."""
    NV = len(chans)
    wt = np.zeros((NV, 128, 128), np.float64)
    wb = np.zeros((NV, 2, 2, 128, 128), np.float64)  # [v, r2, half, (c,i), j]
    wb2 = np.zeros((NV, 2, 2, 128, 64), np.float64)  # [v, r2, sc, (c,i), j']
    wp = np.zeros((128, 2 * NV), np.float64)
    ga = np.zeros((64, NV * 128), np.float64)  # rows r2*32+v, col block v

    for vi, ch in enumerate(chans):
        a1, a2, b0 = a[ch, 1], a[ch, 2], b[ch, 0]
        r = np.sqrt(a2)
        costh = -a1 / (2.0 * r)
        sinth = np.sqrt(max(0.0, 1.0 - costh * costh))
        th = np.arctan2(sinth, costh)

        # impulse response h[m] = b0 * phi(m), phi: homogeneous w/ phi(0)=1
        h = np.zeros(130)
        h[0] = b0
        h[1] = -a1 * b0
        for m in range(2, 130):
            h[m] = -a1 * h[m - 1] - a2 * h[m - 2]

        # Toeplitz lhsT: wt[k, m] = h[m-k] for m >= k
        k_i = np.arange(128)
        d = k_i[None, :] - k_i[:, None]  # [k, m] -> m - k
        wt[vi] = np.where(d >= 0, h[np.clip(d, 0, 129)], 0.0)

        # modal decomposition: A = V S Vinv,
        # V = [[r c, r s],[1,0]], S = r[[c, s],[-s, c]],
        # Vinv = [[0,1],[1/(r s), -c/s]]
        Vinv = np.array([[0.0, 1.0], [1.0 / (r * sinth), -costh / sinth]])

        # wp = W_T[:, [127,126]] @ Vinv.T  (p~ = wp.T @ x_block)
        wp[:, 2 * vi:2 * vi + 2] = wt[vi][:, [127, 126]] @ Vinv.T

        # g[c, n] = r^{n+2} (cos((n+2)th), sin((n+2)th)); block-sparse rows
        ks = np.arange(1, 129)
        rk = r ** (ks + 1.0)
        ga[vi, vi * 128:(vi + 1) * 128] = rk * np.cos((ks + 1) * th)
        ga[32 + vi, vi * 128:(vi + 1) * 128] = rk * np.sin((ks + 1) * th)

        # Mpow[q] = S^(128 q): scaled rotations
        qs = np.arange(0, 193)
        rq = r ** (128.0 * qs)
        ang = 128.0 * qs * th
        Mg = np.zeros((2, 2, 193))
        Mg[0, 0] = rq * np.cos(ang)
        Mg[0, 1] = rq * np.sin(ang)
        Mg[1, 0] = -Mg[0, 1]
        Mg[1, 1] = Mg[0, 0]

        # K-dim packing is c-major: kk = c*64 + i (i = block within chunk)
        ii = np.arange(64)
        jj = np.arange(128)
        j1 = np.arange(64)
        for r2 in range(2):
            for half in range(2):
                dd = jj[None, :] - (64 * half + ii[:, None])
                msk = dd >= 0
                dc = np.clip(dd, 0, 128)
                wb[vi, r2, half, 0:64, :] = np.where(msk, Mg[r2, 0, dc], 0.0)
                wb[vi, r2, half, 64:128, :] = np.where(msk, Mg[r2, 1, dc], 0.0)
            # chunk-2 states directly from p~ of chunks 0/1 (no sl round-trip):
            # power d = (128 + j') - (64*sc + i), j' in 0..63
            for sc in range(2):
                dd2 = 128 + j1[None, :] - 64 * sc - ii[:, None]
                wb2[vi, r2, sc, 0:64, :] = Mg[r2, 0, dd2]
                wb2[vi, r2, sc, 64:128, :] = Mg[r2, 1, dd2]

    # device SBUF layouts, fp16 with global scaling
    return {
        "wt": np.ascontiguousarray(
            (wt / BETA).transpose(1, 0, 2).reshape(128, NV * 128)
        ).astype(np.float16),
        "wb": np.ascontiguousarray(
            wb.transpose(3, 0, 1, 2, 4).reshape(128, NV * 2 * 2 * 128)
        ).astype(np.float16),
        "wb2": np.ascontiguousarray(
            wb2.transpose(3, 0, 1, 2, 4).reshape(128, NV * 2 * 2 * 64)
        ).astype(np.float16),
        "wp": (wp * ALPHA).astype(np.float16),
        "gall": (ga / (ALPHA * BETA)).astype(np.float16),
    }


# --------------------------------------------------------------------------
# device program
# --------------------------------------------------------------------------

def build_nc():
    """Build + compile the single-core Tile program (same on all 8 cores)."""
    import concourse.bass as bass
    import concourse.tile as tile
    from concourse import bacc, mybir

    f32 = mybir.dt.float32
    f16 = mybir.dt.float16

    nc = bacc.Bacc("TRN2", target_bir_lowering=False, debug=False)

    xr_d = nc.dram_tensor("xrhs", [128, NCOL], f16, kind="ExternalInput")
    xr2_d = nc.dram_tensor("xrhs2", [128, NCOL], f16, kind="ExternalInput")
    wt_d = nc.dram_tensor("wt", [128, NVC * 128], f16, kind="ExternalInput")
    wb_d = nc.dram_tensor("wb", [128, NVC * 512], f16, kind="ExternalInput")
    wb2_d = nc.dram_tensor("wb2", [128, NVC * 256], f16, kind="ExternalInput")
    wp_d = nc.dram_tensor("wp", [128, 2 * NVC], f16, kind="ExternalInput")
    ga_d = nc.dram_tensor("gall", [64, NVC * 128], f16, kind="ExternalInput")
    id_d = nc.dram_tensor("ident", [128, 128], f16, kind="ExternalInput")
    out_d = nc.dram_tensor("out", [NVC, 128, NCOL], f16, kind="ExternalOutput")

    with tile.TileContext(nc) as tc:
        with (
            tc.tile_pool(name="const", bufs=1) as const,
            tc.tile_pool(name="work", bufs=1) as work,
            tc.tile_pool(name="rp", bufs=16) as rp_pool,
            tc.tile_pool(name="yout", bufs=4) as yout_pool,
            tc.tile_pool(name="bpsum", bufs=2, space="PSUM") as bpsum,
            tc.tile_pool(name="opsum", bufs=3, space="PSUM") as opsum,
        ):
            # ---- constants into SBUF. Startup-latency aware: xrhs2/wp first
            # (feed p~), then the phase-B tables, then phase-A tables.
            xt2 = const.tile([128, NCOL], f16)
            nc.sync.dma_start(xt2[:], xr2_d[:])
            wp_t = const.tile([128, 2 * NVC], f16)
            nc.sync.dma_start(wp_t[:], wp_d[:])
            # phase-B tables, split per vc-pair/quad so early vcs unblock fast
            wb_t = const.tile([128, NVC * 512], f16)
            wb2_t = const.tile([128, NVC * 256], f16)
            wb_eng = (nc.gpsimd, nc.scalar)
            for q in range(8):
                wb_eng[q % 2].dma_start(
                    wb_t[:, q * 1024:(q + 1) * 1024], wb_d[:, q * 1024:(q + 1) * 1024]
                )
            for q in range(4):
                wb_eng[q % 2].dma_start(
                    wb2_t[:, q * 1024:(q + 1) * 1024],
                    wb2_d[:, q * 1024:(q + 1) * 1024],
                )
            id_t = const.tile([128, 128], f16)
            nc.scalar.dma_start(id_t[:], id_d[:])

            # ---- persistent work tiles
            pp_all = work.tile([32, NCOL], f16)
            ss0_all = work.tile([128, 256], f16)  # cols r2*128 + v*8 + b
            ss1_all = work.tile([64, 256], f16)
            sv_all = work.tile([64, NCOL], f16)  # rows r2*32+v, cols b*192+n
            # zero once, early: covers the per-batch zero-state columns AND
            # the padding rows (16:32, 48:64) the K=64 phase-C rhs reads
            nc.vector.memset(sv_all[:], 0.0)

            # ---- p~ for all 16 vcs: one matmul per 512-col slice (xrhs2)
            pp_copy_eng = (nc.vector, nc.scalar, nc.vector)
            for s in range(3):
                pt = opsum.tile([32, SLW], f32, tag="o")
                nc.tensor.matmul(
                    pt[:], wp_t[:], xt2[:, s * SLW:(s + 1) * SLW],
                    start=True, stop=True,
                )
                eng = pp_copy_eng[s]
                if eng is nc.scalar:
                    eng.copy(pp_all[:, s * SLW:(s + 1) * SLW], pt[:])
                else:
                    eng.tensor_copy(pp_all[:, s * SLW:(s + 1) * SLW], pt[:])

            # ---- phase B: rp loads + s0 matmuls + ss0 copies (all vcs)
            rp_eng = (nc.sync, nc.scalar, nc.gpsimd) * 6
            s0_copy_eng = (nc.vector, nc.scalar)
            rps = []
            for v in range(NVC):
                rp = rp_pool.tile([128, 24], f16, tag="rp", name=f"rp{v}")
                rp_eng[v].dma_start(rp[:], pp_all[2 * v:2 * v + 2, :])
                rps.append(rp)

            for v in range(NVC):
                rp = rps[v]
                s0 = bpsum.tile([128, 16], f32, tag="bp")
                for r2 in (0, 1):
                    base = v * 512 + r2 * 256
                    cs = slice(r2 * 8, r2 * 8 + 8)
                    nc.tensor.matmul(
                        s0[:, cs], wb_t[:, base:base + 128],
                        rp[:, 0:8], start=True, stop=False,
                    )
                    nc.tensor.matmul(
                        s0[:, cs], wb_t[:, base + 128:base + 256],
                        rp[:, 8:16], start=False, stop=True,
                    )
                # ss0_all[:, r2*128 + v*8 + b] <- s0[:, r2*8 + b]
                dst = ss0_all[:].rearrange("p (r w) -> p r w", r=2, w=128)
                src = s0[:].rearrange("p (r b) -> p r b", r=2, b=8)
                eng = s0_copy_eng[v % 2]
                if eng is nc.scalar:
                    eng.copy(dst[:, :, v * 8:(v + 1) * 8], src)
                else:
                    eng.tensor_copy(dst[:, :, v * 8:(v + 1) * 8], src)

            # ---- s1 matmuls + ss1 copies (chunk-2 states straight from p~)
            for v in range(NVC):
                rp = rps[v]
                s1 = bpsum.tile([64, 16], f32, tag="bp")
                for r2 in (0, 1):
                    base = v * 512 + r2 * 256
                    b2 = v * 256 + r2 * 128
                    cs = slice(r2 * 8, r2 * 8 + 8)
                    nc.tensor.matmul(
                        s1[:, cs], wb2_t[:, b2:b2 + 64],
                        rp[:, 0:8], start=True, stop=False,
                    )
                    nc.tensor.matmul(
                        s1[:, cs], wb2_t[:, b2 + 64:b2 + 128],
                        rp[:, 8:16], start=False, stop=False,
                    )
                    nc.tensor.matmul(
                        s1[:, cs], wb_t[:, base:base + 64],
                        rp[:, 16:24], start=False, stop=True,
                    )
                dst = ss1_all[:].rearrange("p (r w) -> p r w", r=2, w=128)
                src = s1[:].rearrange("p (r b) -> p r b", r=2, b=8)
                eng = s0_copy_eng[v % 2]
                if eng is nc.scalar:
                    eng.copy(dst[:, :, v * 8:(v + 1) * 8], src)
                else:
                    eng.tensor_copy(dst[:, :, v * 8:(v + 1) * 8], src)

            # ---- phase-A tables (needed only once sv_all is ready)
            xt = const.tile([128, NCOL], f16)
            nc.sync.dma_start(xt[:], xr_d[:])
            wt_t = const.tile([128, NVC * 128], f16)
            nc.sync.dma_start(wt_t[:], wt_d[:])
            g_t = const.tile([64, NVC * 128], f16)
            nc.gpsimd.dma_start(g_t[:], ga_d[:])

            # ---- per-(batch, component) strided transposes: 16 vcs at once.
            # psv_r[r2] cols: A-blocks b*128 (s_0..s_127), B-blocks 1024+b*64
            psv_r = [bpsum.tile([16, 1536], f16, tag="bp", name=f"psv{r}")
                     for r in range(2)]
            for r2 in range(2):
                for b in range(8):
                    c0 = r2 * 128 + b
                    nc.tensor.transpose(
                        psv_r[r2][:, b * 128:(b + 1) * 128],
                        ss0_all[:, c0:c0 + 121:8], id_t[:],
                    )
                    nc.tensor.transpose(
                        psv_r[r2][:, 1024 + b * 64:1024 + (b + 1) * 64],
                        ss1_all[:, c0:c0 + 121:8], id_t[0:64, 0:64],
                    )
            # sv_all[r2*32+v, b*192+n] = s_{n-1}; col b*192 stays zero
            sv_copy_eng = (nc.vector, nc.scalar)
            for r2 in range(2):
                dst = sv_all[r2 * 32:r2 * 32 + 16, :].rearrange(
                    "p (b n) -> p b n", b=8, n=192
                )
                in1 = psv_r[r2][:, 0:1024].rearrange("p (b n) -> p b n", b=8, n=128)
                in2 = psv_r[r2][:, 1024:1536].rearrange(
                    "p (b n) -> p b n", b=8, n=64
                )[:, :, 0:63]
                eng = sv_copy_eng[r2]
                if eng is nc.scalar:
                    eng.copy(dst[:, :, 1:129], in1)
                    eng.copy(dst[:, :, 129:192], in2)
                else:
                    eng.tensor_copy(dst[:, :, 1:129], in1)
                    eng.tensor_copy(dst[:, :, 129:192], in2)

            # ---- phases A + C per 512-col slice; K=32 block-sparse g matmul
            # accumulates the state correction onto the phase-A PSUM.
            V, A = nc.vector, nc.scalar
            # alternate DVE/ACT (DVE 8/16ths + slight bias via pattern)
            yo_copy_eng = (V, A, V, A, V, A, V, A, V, A, V, A, V, A, V, A)
            e_out = (nc.sync, nc.gpsimd) * 8
            e_out2 = (nc.gpsimd, nc.sync) * 8
            for v in range(NVC):
                yo = yout_pool.tile([128, NCOL], f16, tag="y", name=f"yo{v}")
                for s in range(NSL):
                    sli = slice(s * SLW, (s + 1) * SLW)
                    ps = opsum.tile([128, SLW], f32, tag="o")
                    nc.tensor.matmul(
                        ps[:], g_t[:, v * 128:(v + 1) * 128], sv_all[:, sli],
                        start=True, stop=False,
                    )
                    nc.tensor.matmul(
                        ps[:], wt_t[:, v * 128:(v + 1) * 128], xt[:, sli],
                        start=False, stop=True,
                    )
                    eng = yo_copy_eng[(v * NSL + s) % 16]
                    if eng is nc.scalar:
                        eng.copy(yo[:, sli], ps[:])
                    else:
                        eng.tensor_copy(yo[:, sli], ps[:])
                e_out[v].dma_start(out_d[v, :, 0:768], yo[:, 0:768])
                e_out2[v].dma_start(out_d[v, :, 768:1536], yo[:, 768:1536])

    nc.compile()
    return nc


def _get_program():
    if "nc" not in _PROGRAM_CACHE:
        _PROGRAM_CACHE["nc"] = build_nc()
    return _PROGRAM_CACHE["nc"]


# --------------------------------------------------------------------------
# host driver
# --------------------------------------------------------------------------

def make_in_maps(x, a_coeffs, b_coeffs):
    x = np.asarray(x, np.float32)
    a = np.asarray(a_coeffs, np.float64)
    b = np.asarray(b_coeffs, np.float64)
    xf = x[:, 0, :]

    def to_rhs(x2d):
        xpad = np.zeros((B, TPAD), np.float32)
        xpad[:, :T] = x2d
        return np.ascontiguousarray(
            xpad.reshape(B, NBLK, L).transpose(2, 0, 1).reshape(128, NCOL)
        ).astype(np.float16)

    def to_rhs2(x2d):
        # chunk-interleaved block-major: X2[k, i*24 + c*8 + b]
        #   = xpad[b, (c*64+i)*128 + k]
        xpad = np.zeros((B, TPAD), np.float32)
        xpad[:, :T] = x2d
        return np.ascontiguousarray(
            xpad.reshape(B, 3, 64, L).transpose(3, 2, 1, 0).reshape(128, NCOL)
        ).astype(np.float16)

    Xf = to_rhs(xf)
    Xb = to_rhs(xf[:, ::-1])
    X2f = to_rhs2(xf)
    X2b = to_rhs2(xf[:, ::-1])
    ident = np.eye(128, dtype=np.float16)

    in_maps = []
    for core in range(8):
        fwd = core < 4
        chans = list(range((core % 4) * NVC, (core % 4) * NVC + NVC))
        tabs = _tables_for_channels(a, b, chans)
        in_maps.append(
            {
                "xrhs": Xf if fwd else Xb,
                "xrhs2": X2f if fwd else X2b,
                "ident": ident,
                **tabs,
            }
        )
    return in_maps


def assemble_output(core_outs):
    y = np.zeros((B, 2 * C, T), np.float32)
    for core in range(8):
        o = np.asarray(core_outs[core]).astype(np.float32) * BETA  # [16, 128, 1536]
        o = o.reshape(NVC, 128, B, NBLK).transpose(2, 0, 3, 1).reshape(B, NVC, TPAD)
        if core < 4:
            y[:, core * NVC:(core + 1) * NVC, :] = o[:, :, :T]
        else:
            y[:, C + (core - 4) * NVC:C + (core - 3) * NVC, :] = o[:, :, :T][:, :, ::-1]
    return y


def kernel(x, a_coeffs, b_coeffs, _trace=False):
    from concourse.bass_utils import run_bass_kernel_spmd

    nc = _get_program()
    in_maps = make_in_maps(x, a_coeffs, b_coeffs)
    res = run_bass_kernel_spmd(
        nc, in_maps, core_ids=list(range(8)), trace=_trace
    )
    y = assemble_output([r["out"] for r in res.results])
    if _trace:
        kernel.last_results = res
    return y


# revision 16
# speedup vs baseline: 1.4696x; 1.0778x over previous
"""Bidirectional 2nd-order IIR filter bank (64 channels) on 8 TRN2 NeuronCores.

Algorithm: block-parallel scan over the time axis (same math as the f32r
baseline, restructured for the v1 cost model where a DMA's engine charge is
free-dim bytes x 0.3855ns, min 500ns, on the dispatching engine).
  - T=24000 padded to 24576 = 192 blocks of L=128.
  - Phase A (per channel): zero-state particular solution of every block via a
    lower-triangular-Toeplitz matmul of the impulse response (PE, fp16).
  - Block states in REAL MODAL coordinates z = Vinv @ (y[n], y[n-1]); per-block
    increments p~ from a folded [128,2] matmul (xrhs2 layout).
  - Phase B: second-level scan as lower-block-triangular matmuls, 3 chunks of
    64 blocks; chunk2 adds a K=2 incoming-state matmul (sl).
  - States for ALL 16 filters are transposed per-batch (8 strided PE
    transposes) into ONE [32, NCOL] fp16 sv tile; phase C is then a K=32
    matmul with a host-built block-sparse g_all (rows 2v,2v+1 hold filter v's
    factors) accumulated onto phase A's PSUM. This avoids 16 expensive
    [2, NCOL] per-filter scatter DMAs entirely.
  - Everything on device is fp16 (PSUM accumulation f32) with global
    power-of-2 scaling: wt/beta, wp*alpha, g/(alpha*beta); the PSUM->SBUF
    output copy multiplies by beta. alpha=2^6, beta=2^-13 keeps every fp16
    stage in [1e-4, 200] (validated: rel_l2 ~5.4e-4 vs f64 reference).
Sharding: 128 (direction x channel) independent filters; cores 0-3 forward
channels 0-63, cores 4-7 backward channels 0-63, 16 filters/core, B=8 local.
Output DRAM layout is [vc, n, col] (col = b*192 + block) fp16; the final
transpose to [b, c, t] and f32 cast happen on host in numpy.

Perf notes (v1 cost model): DMA charge = free-bytes x 0.3855ns (x2 if the
min contiguous run < 512B), min 500ns, on the dispatching engine; DMA init
latency ~1.7us does not occupy the engine. Compute-op charge = free-size x
cycle_t + access bubble. Matmul charge = out free-size x 0.4167ns (fp16),
independent of K and partition count -- so thin-K matmuls and strided-AP
transposes are nearly free, and all small DMAs are batched (one sl DMA for
all 16 filters via the (r2, v, b) ss0 column layout).
"""

import sys

import numpy as np

if "/opt/trn_rl_repo" not in sys.path:
    sys.path.insert(0, "/opt/trn_rl_repo")

T = 24000
B = 8
C = 64
L = 128
NBLK = 192
TPAD = NBLK * L  # 24576
NCOL = B * NBLK  # 1536
NVC = 16  # filters per core
NSL = 3  # 512-wide column slices
SLW = 512

ALPHA = 2.0 ** 6
BETA = 2.0 ** -13

_PROGRAM_CACHE = {}


# --------------------------------------------------------------------------
# host-side table construction (float64 -> float16, device SBUF layouts)
# --------------------------------------------------------------------------

def _tables_for_channels(a, b, chans):
    """Per-core weight tables for 16 channels. a,b: [64,3] float64."""
    NV = len(chans)
    wt = np.zeros((NV, 128, 128), np.float64)
    wb = np.zeros((NV, 2, 2, 128, 128), np.float64)  # [v, r2, half, (c,i), j]
    wb2 = np.zeros((NV, 2, 2, 128, 64), np.float64)  # [v, r2, sc, (c,i), j']
    wp = np.zeros((128, 2 * NV), np.float64)
    ga = np.zeros((64, NV * 128), np.float64)  # rows r2*32+v, col block v

    for vi, ch in enumerate(chans):
        a1, a2, b0 = a[ch, 1], a[ch, 2], b[ch, 0]
        r = np.sqrt(a2)
        costh = -a1 / (2.0 * r)
        sinth = np.sqrt(max(0.0, 1.0 - costh * costh))
        th = np.arctan2(sinth, costh)

        # impulse response h[m] = b0 * phi(m), phi: homogeneous w/ phi(0)=1
        h = np.zeros(130)
        h[0] = b0
        h[1] = -a1 * b0
        for m in range(2, 130):
            h[m] = -a1 * h[m - 1] - a2 * h[m - 2]

        # Toeplitz lhsT: wt[k, m] = h[m-k] for m >= k
        k_i = np.arange(128)
        d = k_i[None, :] - k_i[:, None]  # [k, m] -> m - k
        wt[vi] = np.where(d >= 0, h[np.clip(d, 0, 129)], 0.0)

        # modal decomposition: A = V S Vinv,
        # V = [[r c, r s],[1,0]], S = r[[c, s],[-s, c]],
        # Vinv = [[0,1],[1/(r s), -c/s]]
        Vinv = np.array([[0.0, 1.0], [1.0 / (r * sinth), -costh / sinth]])

        # wp = W_T[:, [127,126]] @ Vinv.T  (p~ = wp.T @ x_block)
        wp[:, 2 * vi:2 * vi + 2] = wt[vi][:, [127, 126]] @ Vinv.T

        # g[c, n] = r^{n+2} (cos((n+2)th), sin((n+2)th)); block-sparse rows
        ks = np.arange(1, 129)
        rk = r ** (ks + 1.0)
        ga[vi, vi * 128:(vi + 1) * 128] = rk * np.cos((ks + 1) * th)
        ga[32 + vi, vi * 128:(vi + 1) * 128] = rk * np.sin((ks + 1) * th)

        # Mpow[q] = S^(128 q): scaled rotations
        qs = np.arange(0, 193)
        rq = r ** (128.0 * qs)
        ang = 128.0 * qs * th
        Mg = np.zeros((2, 2, 193))
        Mg[0, 0] = rq * np.cos(ang)
        Mg[0, 1] = rq * np.sin(ang)
        Mg[1, 0] = -Mg[0, 1]
        Mg[1, 1] = Mg[0, 0]

        # K-dim packing is c-major: kk = c*64 + i (i = block within chunk)
        ii = np.arange(64)
        jj = np.arange(128)
        j1 = np.arange(64)
        for r2 in range(2):
            for half in range(2):
                dd = jj[None, :] - (64 * half + ii[:, None])
                msk = dd >= 0
                dc = np.clip(dd, 0, 128)
                wb[vi, r2, half, 0:64, :] = np.where(msk, Mg[r2, 0, dc], 0.0)
                wb[vi, r2, half, 64:128, :] = np.where(msk, Mg[r2, 1, dc], 0.0)
            # chunk-2 states directly from p~ of chunks 0/1 (no sl round-trip):
            # power d = (128 + j') - (64*sc + i), j' in 0..63
            for sc in range(2):
                dd2 = 128 + j1[None, :] - 64 * sc - ii[:, None]
                wb2[vi, r2, sc, 0:64, :] = Mg[r2, 0, dd2]
                wb2[vi, r2, sc, 64:128, :] = Mg[r2, 1, dd2]

    # device SBUF layouts, fp16 with global scaling
    return {
        "wt": np.ascontiguousarray(
            (wt / BETA).transpose(1, 0, 2).reshape(128, NV * 128)
        ).astype(np.float16),
        "wb": np.ascontiguousarray(
            wb.transpose(3, 0, 1, 2, 4).reshape(128, NV * 2 * 2 * 128)
        ).astype(np.float16),
        "wb2": np.ascontiguousarray(
            wb2.transpose(3, 0, 1, 2, 4).reshape(128, NV * 2 * 2 * 64)
        ).astype(np.float16),
        "wp": (wp * ALPHA).astype(np.float16),
        "gall": (ga / (ALPHA * BETA)).astype(np.float16),
    }


# --------------------------------------------------------------------------
# device program
# --------------------------------------------------------------------------

def build_nc():
    """Build + compile the single-core Tile program (same on all 8 cores)."""
    import concourse.bass as bass
    import concourse.tile as tile
    from concourse import bacc, mybir

    f32 = mybir.dt.float32
    f16 = mybir.dt.float16

    nc = bacc.Bacc("TRN2", target_bir_lowering=False, debug=False)

    xr_d = nc.dram_tensor("xrhs", [128, NCOL], f16, kind="ExternalInput")
    xr2_d = nc.dram_tensor("xrhs2", [128, NCOL], f16, kind="ExternalInput")
    wt_d = nc.dram_tensor("wt", [128, NVC * 128], f16, kind="ExternalInput")
    wb_d = nc.dram_tensor("wb", [128, NVC * 512], f16, kind="ExternalInput")
    wb2_d = nc.dram_tensor("wb2", [128, NVC * 256], f16, kind="ExternalInput")
    wp_d = nc.dram_tensor("wp", [128, 2 * NVC], f16, kind="ExternalInput")
    ga_d = nc.dram_tensor("gall", [64, NVC * 128], f16, kind="ExternalInput")
    id_d = nc.dram_tensor("ident", [128, 128], f16, kind="ExternalInput")
    out_d = nc.dram_tensor("out", [NVC, 128, NCOL], f16, kind="ExternalOutput")

    with tile.TileContext(nc) as tc:
        with (
            tc.tile_pool(name="const", bufs=1) as const,
            tc.tile_pool(name="work", bufs=1) as work,
            tc.tile_pool(name="rp", bufs=16) as rp_pool,
            tc.tile_pool(name="yout", bufs=4) as yout_pool,
            tc.tile_pool(name="bpsum", bufs=4, space="PSUM") as bpsum,
            tc.tile_pool(name="opsum", bufs=3, space="PSUM") as opsum,
        ):
            # ---- constants into SBUF. Startup-latency aware: xrhs2/wp first
            # (feed p~), then the phase-B tables, then phase-A tables.
            xt2 = const.tile([128, NCOL], f16)
            nc.sync.dma_start(xt2[:], xr2_d[:])
            wp_t = const.tile([128, 2 * NVC], f16)
            nc.sync.dma_start(wp_t[:], wp_d[:])
            # phase-B tables, split per vc-pair/quad so early vcs unblock fast
            wb_t = const.tile([128, NVC * 512], f16)
            wb2_t = const.tile([128, NVC * 256], f16)
            wb_eng = (nc.gpsimd, nc.scalar)
            for q in range(8):
                wb_eng[q % 2].dma_start(
                    wb_t[:, q * 1024:(q + 1) * 1024], wb_d[:, q * 1024:(q + 1) * 1024]
                )
            for q in range(4):
                wb_eng[q % 2].dma_start(
                    wb2_t[:, q * 1024:(q + 1) * 1024],
                    wb2_d[:, q * 1024:(q + 1) * 1024],
                )
            id_t = const.tile([128, 128], f16)
            nc.scalar.dma_start(id_t[:], id_d[:])

            # ---- persistent work tiles
            pp_all = work.tile([32, NCOL], f16)
            ss0_all = work.tile([128, 256], f16)  # cols r2*128 + v*8 + b
            ss1_all = work.tile([64, 256], f16)
            sv_all = work.tile([64, NCOL], f16)  # rows r2*32+v, cols b*192+n
            # zero once, early: covers the per-batch zero-state columns AND
            # the padding rows (16:32, 48:64) the K=64 phase-C rhs reads
            nc.vector.memset(sv_all[:], 0.0)

            # ---- p~ for all 16 vcs: one matmul per 512-col slice (xrhs2)
            pp_copy_eng = (nc.vector, nc.scalar, nc.vector)
            for s in range(3):
                pt = opsum.tile([32, SLW], f32, tag="o")
                nc.tensor.matmul(
                    pt[:], wp_t[:], xt2[:, s * SLW:(s + 1) * SLW],
                    start=True, stop=True,
                )
                eng = pp_copy_eng[s]
                if eng is nc.scalar:
                    eng.copy(pp_all[:, s * SLW:(s + 1) * SLW], pt[:])
                else:
                    eng.tensor_copy(pp_all[:, s * SLW:(s + 1) * SLW], pt[:])

            # ---- phase B: rp loads + s0 matmuls + ss0 copies (all vcs)
            rp_eng = (nc.sync, nc.scalar, nc.gpsimd) * 6
            s0_copy_eng = (nc.vector, nc.scalar)
            rps = []
            for v in range(NVC):
                rp = rp_pool.tile([128, 24], f16, tag="rp", name=f"rp{v}")
                rp_eng[v].dma_start(rp[:], pp_all[2 * v:2 * v + 2, :])
                rps.append(rp)

            for v in range(NVC):
                rp = rps[v]
                s0 = bpsum.tile([128, 16], f32, tag="bp")
                for r2 in (0, 1):
                    base = v * 512 + r2 * 256
                    cs = slice(r2 * 8, r2 * 8 + 8)
                    nc.tensor.matmul(
                        s0[:, cs], wb_t[:, base:base + 128],
                        rp[:, 0:8], start=True, stop=False,
                    )
                    nc.tensor.matmul(
                        s0[:, cs], wb_t[:, base + 128:base + 256],
                        rp[:, 8:16], start=False, stop=True,
                    )
                # ss0_all[:, r2*128 + v*8 + b] <- s0[:, r2*8 + b]
                dst = ss0_all[:].rearrange("p (r w) -> p r w", r=2, w=128)
                src = s0[:].rearrange("p (r b) -> p r b", r=2, b=8)
                eng = s0_copy_eng[v % 2]
                if eng is nc.scalar:
                    eng.copy(dst[:, :, v * 8:(v + 1) * 8], src)
                else:
                    eng.tensor_copy(dst[:, :, v * 8:(v + 1) * 8], src)

            # ---- s1 matmuls + ss1 copies (chunk-2 states straight from p~)
            for v in range(NVC):
                rp = rps[v]
                s1 = bpsum.tile([64, 16], f32, tag="bp")
                for r2 in (0, 1):
                    base = v * 512 + r2 * 256
                    b2 = v * 256 + r2 * 128
                    cs = slice(r2 * 8, r2 * 8 + 8)
                    nc.tensor.matmul(
                        s1[:, cs], wb2_t[:, b2:b2 + 64],
                        rp[:, 0:8], start=True, stop=False,
                    )
                    nc.tensor.matmul(
                        s1[:, cs], wb2_t[:, b2 + 64:b2 + 128],
                        rp[:, 8:16], start=False, stop=False,
                    )
                    nc.tensor.matmul(
                        s1[:, cs], wb_t[:, base:base + 64],
                        rp[:, 16:24], start=False, stop=True,
                    )
                dst = ss1_all[:].rearrange("p (r w) -> p r w", r=2, w=128)
                src = s1[:].rearrange("p (r b) -> p r b", r=2, b=8)
                eng = s0_copy_eng[v % 2]
                if eng is nc.scalar:
                    eng.copy(dst[:, :, v * 8:(v + 1) * 8], src)
                else:
                    eng.tensor_copy(dst[:, :, v * 8:(v + 1) * 8], src)

            # ---- phase-A tables (needed only once sv_all is ready)
            xt = const.tile([128, NCOL], f16)
            nc.sync.dma_start(xt[:], xr_d[:])
            wt_t = const.tile([128, NVC * 128], f16)
            nc.sync.dma_start(wt_t[:], wt_d[:])
            g_t = const.tile([64, NVC * 128], f16)
            nc.gpsimd.dma_start(g_t[:], ga_d[:])

            # ---- per-(batch, component) strided transposes: 16 vcs at once.
            # psva[r2]: A-blocks b*128 (s_0..s_127); psvb[r2]: B-blocks b*64
            psva = [bpsum.tile([16, 1024], f16, tag="bp", name=f"psva{r}")
                    for r in range(2)]
            psvb = [bpsum.tile([16, 512], f16, tag="bp", name=f"psvb{r}")
                    for r in range(2)]
            for r2 in range(2):
                for b in range(8):
                    c0 = r2 * 128 + b
                    nc.tensor.transpose(
                        psva[r2][:, b * 128:(b + 1) * 128],
                        ss0_all[:, c0:c0 + 121:8], id_t[:],
                    )
                    nc.tensor.transpose(
                        psvb[r2][:, b * 64:(b + 1) * 64],
                        ss1_all[:, c0:c0 + 121:8], id_t[0:64, 0:64],
                    )
            # sv_all[r2*32+v, b*192+n] = s_{n-1}; col b*192 stays zero
            sv_copy_eng = (nc.vector, nc.scalar)
            for r2 in range(2):
                dst = sv_all[r2 * 32:r2 * 32 + 16, :].rearrange(
                    "p (b n) -> p b n", b=8, n=192
                )
                in1 = psva[r2][:].rearrange("p (b n) -> p b n", b=8, n=128)
                in2 = psvb[r2][:].rearrange(
                    "p (b n) -> p b n", b=8, n=64
                )[:, :, 0:63]
                eng = sv_copy_eng[r2]
                if eng is nc.scalar:
                    eng.copy(dst[:, :, 1:129], in1)
                    eng.copy(dst[:, :, 129:192], in2)
                else:
                    eng.tensor_copy(dst[:, :, 1:129], in1)
                    eng.tensor_copy(dst[:, :, 129:192], in2)

            # ---- phases A + C per 512-col slice; K=32 block-sparse g matmul
            # accumulates the state correction onto the phase-A PSUM.
            V, A = nc.vector, nc.scalar
            # alternate DVE/ACT (DVE 8/16ths + slight bias via pattern)
            yo_copy_eng = (V, A, V, A, V, A, V, A, V, A, V, A, V, A, V, A)
            e_out = (nc.sync, nc.gpsimd) * 8
            e_out2 = (nc.gpsimd, nc.sync) * 8
            for v in range(NVC):
                yo = yout_pool.tile([128, NCOL], f16, tag="y", name=f"yo{v}")
                for s in range(NSL):
                    sli = slice(s * SLW, (s + 1) * SLW)
                    ps = opsum.tile([128, SLW], f32, tag="o")
                    nc.tensor.matmul(
                        ps[:], wt_t[:, v * 128:(v + 1) * 128], xt[:, sli],
                        start=True, stop=False,
                    )
                    nc.tensor.matmul(
                        ps[:], g_t[:, v * 128:(v + 1) * 128], sv_all[:, sli],
                        start=False, stop=True,
                    )
                    eng = yo_copy_eng[(v * NSL + s) % 16]
                    if eng is nc.scalar:
                        eng.copy(yo[:, sli], ps[:])
                    else:
                        eng.tensor_copy(yo[:, sli], ps[:])
                e_out[v].dma_start(out_d[v, :, 0:768], yo[:, 0:768])
                e_out2[v].dma_start(out_d[v, :, 768:1536], yo[:, 768:1536])

    nc.compile()
    return nc


def _get_program():
    if "nc" not in _PROGRAM_CACHE:
        _PROGRAM_CACHE["nc"] = build_nc()
    return _PROGRAM_CACHE["nc"]


# --------------------------------------------------------------------------
# host driver
# --------------------------------------------------------------------------

def make_in_maps(x, a_coeffs, b_coeffs):
    x = np.asarray(x, np.float32)
    a = np.asarray(a_coeffs, np.float64)
    b = np.asarray(b_coeffs, np.float64)
    xf = x[:, 0, :]

    def to_rhs(x2d):
        xpad = np.zeros((B, TPAD), np.float32)
        xpad[:, :T] = x2d
        return np.ascontiguousarray(
            xpad.reshape(B, NBLK, L).transpose(2, 0, 1).reshape(128, NCOL)
        ).astype(np.float16)

    def to_rhs2(x2d):
        # chunk-interleaved block-major: X2[k, i*24 + c*8 + b]
        #   = xpad[b, (c*64+i)*128 + k]
        xpad = np.zeros((B, TPAD), np.float32)
        xpad[:, :T] = x2d
        return np.ascontiguousarray(
            xpad.reshape(B, 3, 64, L).transpose(3, 2, 1, 0).reshape(128, NCOL)
        ).astype(np.float16)

    Xf = to_rhs(xf)
    Xb = to_rhs(xf[:, ::-1])
    X2f = to_rhs2(xf)
    X2b = to_rhs2(xf[:, ::-1])
    ident = np.eye(128, dtype=np.float16)

    in_maps = []
    for core in range(8):
        fwd = core < 4
        chans = list(range((core % 4) * NVC, (core % 4) * NVC + NVC))
        tabs = _tables_for_channels(a, b, chans)
        in_maps.append(
            {
                "xrhs": Xf if fwd else Xb,
                "xrhs2": X2f if fwd else X2b,
                "ident": ident,
                **tabs,
            }
        )
    return in_maps


def assemble_output(core_outs):
    y = np.zeros((B, 2 * C, T), np.float32)
    for core in range(8):
        o = np.asarray(core_outs[core]).astype(np.float32) * BETA  # [16, 128, 1536]
        o = o.reshape(NVC, 128, B, NBLK).transpose(2, 0, 3, 1).reshape(B, NVC, TPAD)
        if core < 4:
            y[:, core * NVC:(core + 1) * NVC, :] = o[:, :, :T]
        else:
            y[:, C + (core - 4) * NVC:C + (core - 3) * NVC, :] = o[:, :, :T][:, :, ::-1]
    return y


def kernel(x, a_coeffs, b_coeffs, _trace=False):
    from concourse.bass_utils import run_bass_kernel_spmd

    nc = _get_program()
    in_maps = make_in_maps(x, a_coeffs, b_coeffs)
    res = run_bass_kernel_spmd(
        nc, in_maps, core_ids=list(range(8)), trace=_trace
    )
    y = assemble_output([r["out"] for r in res.results])
    if _trace:
        kernel.last_results = res
    return y


# revision 17
# speedup vs baseline: 1.4723x; 1.0018x over previous
"""Bidirectional 2nd-order IIR filter bank (64 channels) on 8 TRN2 NeuronCores.

Algorithm: block-parallel scan over the time axis (same math as the f32r
baseline, restructured for the v1 cost model where a DMA's engine charge is
free-dim bytes x 0.3855ns, min 500ns, on the dispatching engine).
  - T=24000 padded to 24576 = 192 blocks of L=128.
  - Phase A (per channel): zero-state particular solution of every block via a
    lower-triangular-Toeplitz matmul of the impulse response (PE, fp16).
  - Block states in REAL MODAL coordinates z = Vinv @ (y[n], y[n-1]); per-block
    increments p~ from a folded [128,2] matmul (xrhs2 layout).
  - Phase B: second-level scan as lower-block-triangular matmuls, 3 chunks of
    64 blocks; chunk2 adds a K=2 incoming-state matmul (sl).
  - States for ALL 16 filters are transposed per-batch (8 strided PE
    transposes) into ONE [32, NCOL] fp16 sv tile; phase C is then a K=32
    matmul with a host-built block-sparse g_all (rows 2v,2v+1 hold filter v's
    factors) accumulated onto phase A's PSUM. This avoids 16 expensive
    [2, NCOL] per-filter scatter DMAs entirely.
  - Everything on device is fp16 (PSUM accumulation f32) with global
    power-of-2 scaling: wt/beta, wp*alpha, g/(alpha*beta); the PSUM->SBUF
    output copy multiplies by beta. alpha=2^6, beta=2^-13 keeps every fp16
    stage in [1e-4, 200] (validated: rel_l2 ~5.4e-4 vs f64 reference).
Sharding: 128 (direction x channel) independent filters; cores 0-3 forward
channels 0-63, cores 4-7 backward channels 0-63, 16 filters/core, B=8 local.
Output DRAM layout is [vc, n, col] (col = b*192 + block) fp16; the final
transpose to [b, c, t] and f32 cast happen on host in numpy.

Perf notes (v1 cost model): DMA charge = free-bytes x 0.3855ns (x2 if the
min contiguous run < 512B), min 500ns, on the dispatching engine; DMA init
latency ~1.7us does not occupy the engine. Compute-op charge = free-size x
cycle_t + access bubble. Matmul charge = out free-size x 0.4167ns (fp16),
independent of K and partition count -- so thin-K matmuls and strided-AP
transposes are nearly free, and all small DMAs are batched (one sl DMA for
all 16 filters via the (r2, v, b) ss0 column layout).
"""

import sys

import numpy as np

if "/opt/trn_rl_repo" not in sys.path:
    sys.path.insert(0, "/opt/trn_rl_repo")

T = 24000
B = 8
C = 64
L = 128
NBLK = 192
TPAD = NBLK * L  # 24576
NCOL = B * NBLK  # 1536
NVC = 16  # filters per core
NSL = 3  # 512-wide column slices
SLW = 512

ALPHA = 2.0 ** 6
BETA = 2.0 ** -13

_PROGRAM_CACHE = {}


# --------------------------------------------------------------------------
# host-side table construction (float64 -> float16, device SBUF layouts)
# --------------------------------------------------------------------------

def _tables_for_channels(a, b, chans):
    """Per-core weight tables for 16 channels. a,b: [64,3] float64."""
    NV = len(chans)
    wt = np.zeros((NV, 128, 128), np.float64)
    wb = np.zeros((NV, 2, 2, 128, 128), np.float64)  # [v, r2, half, (c,i), j]
    wb2 = np.zeros((NV, 2, 2, 128, 64), np.float64)  # [v, r2, sc, (c,i), j']
    wp = np.zeros((128, 2 * NV), np.float64)
    ga = np.zeros((64, NV * 128), np.float64)  # rows r2*32+v, col block v

    for vi, ch in enumerate(chans):
        a1, a2, b0 = a[ch, 1], a[ch, 2], b[ch, 0]
        r = np.sqrt(a2)
        costh = -a1 / (2.0 * r)
        sinth = np.sqrt(max(0.0, 1.0 - costh * costh))
        th = np.arctan2(sinth, costh)

        # impulse response h[m] = b0 * phi(m), phi: homogeneous w/ phi(0)=1
        h = np.zeros(130)
        h[0] = b0
        h[1] = -a1 * b0
        for m in range(2, 130):
            h[m] = -a1 * h[m - 1] - a2 * h[m - 2]

        # Toeplitz lhsT: wt[k, m] = h[m-k] for m >= k
        k_i = np.arange(128)
        d = k_i[None, :] - k_i[:, None]  # [k, m] -> m - k
        wt[vi] = np.where(d >= 0, h[np.clip(d, 0, 129)], 0.0)

        # modal decomposition: A = V S Vinv,
        # V = [[r c, r s],[1,0]], S = r[[c, s],[-s, c]],
        # Vinv = [[0,1],[1/(r s), -c/s]]
        Vinv = np.array([[0.0, 1.0], [1.0 / (r * sinth), -costh / sinth]])

        # wp = W_T[:, [127,126]] @ Vinv.T  (p~ = wp.T @ x_block)
        wp[:, 2 * vi:2 * vi + 2] = wt[vi][:, [127, 126]] @ Vinv.T

        # g[c, n] = r^{n+2} (cos((n+2)th), sin((n+2)th)); block-sparse rows
        ks = np.arange(1, 129)
        rk = r ** (ks + 1.0)
        ga[vi, vi * 128:(vi + 1) * 128] = rk * np.cos((ks + 1) * th)
        ga[32 + vi, vi * 128:(vi + 1) * 128] = rk * np.sin((ks + 1) * th)

        # Mpow[q] = S^(128 q): scaled rotations
        qs = np.arange(0, 193)
        rq = r ** (128.0 * qs)
        ang = 128.0 * qs * th
        Mg = np.zeros((2, 2, 193))
        Mg[0, 0] = rq * np.cos(ang)
        Mg[0, 1] = rq * np.sin(ang)
        Mg[1, 0] = -Mg[0, 1]
        Mg[1, 1] = Mg[0, 0]

        # K-dim packing is c-major: kk = c*64 + i (i = block within chunk)
        ii = np.arange(64)
        jj = np.arange(128)
        j1 = np.arange(64)
        for r2 in range(2):
            for half in range(2):
                dd = jj[None, :] - (64 * half + ii[:, None])
                msk = dd >= 0
                dc = np.clip(dd, 0, 128)
                wb[vi, r2, half, 0:64, :] = np.where(msk, Mg[r2, 0, dc], 0.0)
                wb[vi, r2, half, 64:128, :] = np.where(msk, Mg[r2, 1, dc], 0.0)
            # chunk-2 states directly from p~ of chunks 0/1 (no sl round-trip):
            # power d = (128 + j') - (64*sc + i), j' in 0..63
            for sc in range(2):
                dd2 = 128 + j1[None, :] - 64 * sc - ii[:, None]
                wb2[vi, r2, sc, 0:64, :] = Mg[r2, 0, dd2]
                wb2[vi, r2, sc, 64:128, :] = Mg[r2, 1, dd2]

    # device SBUF layouts, fp16 with global scaling
    return {
        "wt": np.ascontiguousarray(
            (wt / BETA).transpose(1, 0, 2).reshape(128, NV * 128)
        ).astype(np.float16),
        "wb": np.ascontiguousarray(
            wb.transpose(3, 0, 1, 2, 4).reshape(128, NV * 2 * 2 * 128)
        ).astype(np.float16),
        "wb2": np.ascontiguousarray(
            wb2.transpose(3, 0, 1, 2, 4).reshape(128, NV * 2 * 2 * 64)
        ).astype(np.float16),
        "wp": (wp * ALPHA).astype(np.float16),
        "gall": (ga / (ALPHA * BETA)).astype(np.float16),
    }


# --------------------------------------------------------------------------
# device program
# --------------------------------------------------------------------------

def build_nc():
    """Build + compile the single-core Tile program (same on all 8 cores)."""
    import concourse.bass as bass
    import concourse.tile as tile
    from concourse import bacc, mybir

    f32 = mybir.dt.float32
    f16 = mybir.dt.float16

    nc = bacc.Bacc("TRN2", target_bir_lowering=False, debug=False)

    xr_d = nc.dram_tensor("xrhs", [128, NCOL], f16, kind="ExternalInput")
    xr2_d = nc.dram_tensor("xrhs2", [128, NCOL], f16, kind="ExternalInput")
    wt_d = nc.dram_tensor("wt", [128, NVC * 128], f16, kind="ExternalInput")
    wb_d = nc.dram_tensor("wb", [128, NVC * 512], f16, kind="ExternalInput")
    wb2_d = nc.dram_tensor("wb2", [128, NVC * 256], f16, kind="ExternalInput")
    wp_d = nc.dram_tensor("wp", [128, 2 * NVC], f16, kind="ExternalInput")
    ga_d = nc.dram_tensor("gall", [64, NVC * 128], f16, kind="ExternalInput")
    id_d = nc.dram_tensor("ident", [128, 128], f16, kind="ExternalInput")
    out_d = nc.dram_tensor("out", [NVC, 128, NCOL], f16, kind="ExternalOutput")

    with tile.TileContext(nc) as tc:
        with (
            tc.tile_pool(name="const", bufs=1) as const,
            tc.tile_pool(name="work", bufs=1) as work,
            tc.tile_pool(name="rp", bufs=16) as rp_pool,
            tc.tile_pool(name="yout", bufs=4) as yout_pool,
            tc.tile_pool(name="bpsum", bufs=4, space="PSUM") as bpsum,
            tc.tile_pool(name="opsum", bufs=3, space="PSUM") as opsum,
        ):
            # ---- constants into SBUF. Startup-latency aware: xrhs2/wp first
            # (feed p~), then the phase-B tables, then phase-A tables.
            xt2 = const.tile([128, NCOL], f16)
            nc.sync.dma_start(xt2[:, 0:768], xr2_d[:, 0:768])
            nc.scalar.dma_start(xt2[:, 768:1536], xr2_d[:, 768:1536])
            wp_t = const.tile([128, 2 * NVC], f16)
            nc.sync.dma_start(wp_t[:], wp_d[:])
            # phase-B tables, split per vc-pair/quad so early vcs unblock fast
            wb_t = const.tile([128, NVC * 512], f16)
            wb2_t = const.tile([128, NVC * 256], f16)
            wb_eng = (nc.gpsimd, nc.gpsimd, nc.scalar, nc.gpsimd)
            for q in range(8):
                wb_eng[q % 4].dma_start(
                    wb_t[:, q * 1024:(q + 1) * 1024], wb_d[:, q * 1024:(q + 1) * 1024]
                )
            for q in range(4):
                wb_eng[(q + 2) % 4].dma_start(
                    wb2_t[:, q * 1024:(q + 1) * 1024],
                    wb2_d[:, q * 1024:(q + 1) * 1024],
                )
            id_t = const.tile([128, 128], f16)
            nc.scalar.dma_start(id_t[:], id_d[:])

            # ---- persistent work tiles
            pp_all = work.tile([32, NCOL], f16)
            ss0_all = work.tile([128, 256], f16)  # cols r2*128 + v*8 + b
            ss1_all = work.tile([64, 256], f16)
            sv_all = work.tile([64, NCOL], f16)  # rows r2*32+v, cols b*192+n
            # zero once, early: covers the per-batch zero-state columns AND
            # the padding rows (16:32, 48:64) the K=64 phase-C rhs reads
            nc.vector.memset(sv_all[:], 0.0)

            # ---- p~ for all 16 vcs: one matmul per 512-col slice (xrhs2)
            pp_copy_eng = (nc.vector, nc.scalar, nc.vector)
            for s in range(3):
                pt = opsum.tile([32, SLW], f32, tag="o")
                nc.tensor.matmul(
                    pt[:], wp_t[:], xt2[:, s * SLW:(s + 1) * SLW],
                    start=True, stop=True,
                )
                eng = pp_copy_eng[s]
                if eng is nc.scalar:
                    eng.copy(pp_all[:, s * SLW:(s + 1) * SLW], pt[:])
                else:
                    eng.tensor_copy(pp_all[:, s * SLW:(s + 1) * SLW], pt[:])

            # ---- phase B: rp loads + s0 matmuls + ss0 copies (all vcs)
            rp_eng = (nc.sync, nc.scalar, nc.gpsimd) * 6
            s0_copy_eng = (nc.vector, nc.vector, nc.vector, nc.scalar)
            rps = []
            for v in range(NVC):
                rp = rp_pool.tile([128, 24], f16, tag="rp", name=f"rp{v}")
                rp_eng[v].dma_start(rp[:], pp_all[2 * v:2 * v + 2, :])
                rps.append(rp)

            for v in range(NVC):
                rp = rps[v]
                s0 = bpsum.tile([128, 16], f32, tag="bp")
                for r2 in (0, 1):
                    base = v * 512 + r2 * 256
                    cs = slice(r2 * 8, r2 * 8 + 8)
                    nc.tensor.matmul(
                        s0[:, cs], wb_t[:, base:base + 128],
                        rp[:, 0:8], start=True, stop=False,
                    )
                    nc.tensor.matmul(
                        s0[:, cs], wb_t[:, base + 128:base + 256],
                        rp[:, 8:16], start=False, stop=True,
                    )
                # ss0_all[:, r2*128 + v*8 + b] <- s0[:, r2*8 + b]
                dst = ss0_all[:].rearrange("p (r w) -> p r w", r=2, w=128)
                src = s0[:].rearrange("p (r b) -> p r b", r=2, b=8)
                eng = s0_copy_eng[v % 4]
                if eng is nc.scalar:
                    eng.copy(dst[:, :, v * 8:(v + 1) * 8], src)
                else:
                    eng.tensor_copy(dst[:, :, v * 8:(v + 1) * 8], src)

            # ---- s1 matmuls + ss1 copies (chunk-2 states straight from p~)
            for v in range(NVC):
                rp = rps[v]
                s1 = bpsum.tile([64, 16], f32, tag="bp")
                for r2 in (0, 1):
                    base = v * 512 + r2 * 256
                    b2 = v * 256 + r2 * 128
                    cs = slice(r2 * 8, r2 * 8 + 8)
                    nc.tensor.matmul(
                        s1[:, cs], wb2_t[:, b2:b2 + 64],
                        rp[:, 0:8], start=True, stop=False,
                    )
                    nc.tensor.matmul(
                        s1[:, cs], wb2_t[:, b2 + 64:b2 + 128],
                        rp[:, 8:16], start=False, stop=False,
                    )
                    nc.tensor.matmul(
                        s1[:, cs], wb_t[:, base:base + 64],
                        rp[:, 16:24], start=False, stop=True,
                    )
                dst = ss1_all[:].rearrange("p (r w) -> p r w", r=2, w=128)
                src = s1[:].rearrange("p (r b) -> p r b", r=2, b=8)
                eng = s0_copy_eng[v % 4]
                if eng is nc.scalar:
                    eng.copy(dst[:, :, v * 8:(v + 1) * 8], src)
                else:
                    eng.tensor_copy(dst[:, :, v * 8:(v + 1) * 8], src)

            # ---- phase-A tables (needed only once sv_all is ready)
            xt = const.tile([128, NCOL], f16)
            nc.sync.dma_start(xt[:], xr_d[:])
            wt_t = const.tile([128, NVC * 128], f16)
            nc.sync.dma_start(wt_t[:], wt_d[:])
            g_t = const.tile([64, NVC * 128], f16)
            nc.gpsimd.dma_start(g_t[:], ga_d[:])

            # ---- per-(batch, component) strided transposes: 16 vcs at once.
            # psva[r2]: A-blocks b*128 (s_0..s_127); psvb[r2]: B-blocks b*64
            psva = [bpsum.tile([16, 1024], f16, tag="bp", name=f"psva{r}")
                    for r in range(2)]
            psvb = [bpsum.tile([16, 512], f16, tag="bp", name=f"psvb{r}")
                    for r in range(2)]
            for r2 in range(2):
                for b in range(8):
                    c0 = r2 * 128 + b
                    nc.tensor.transpose(
                        psva[r2][:, b * 128:(b + 1) * 128],
                        ss0_all[:, c0:c0 + 121:8], id_t[:],
                    )
                    nc.tensor.transpose(
                        psvb[r2][:, b * 64:(b + 1) * 64],
                        ss1_all[:, c0:c0 + 121:8], id_t[0:64, 0:64],
                    )
            # sv_all[r2*32+v, b*192+n] = s_{n-1}; col b*192 stays zero
            sv_copy_eng = (nc.vector, nc.scalar)
            for r2 in range(2):
                dst = sv_all[r2 * 32:r2 * 32 + 16, :].rearrange(
                    "p (b n) -> p b n", b=8, n=192
                )
                in1 = psva[r2][:].rearrange("p (b n) -> p b n", b=8, n=128)
                in2 = psvb[r2][:].rearrange(
                    "p (b n) -> p b n", b=8, n=64
                )[:, :, 0:63]
                eng = sv_copy_eng[r2]
                if eng is nc.scalar:
                    eng.copy(dst[:, :, 1:129], in1)
                    eng.copy(dst[:, :, 129:192], in2)
                else:
                    eng.tensor_copy(dst[:, :, 1:129], in1)
                    eng.tensor_copy(dst[:, :, 129:192], in2)

            # ---- phases A + C per 512-col slice; K=32 block-sparse g matmul
            # accumulates the state correction onto the phase-A PSUM.
            V, A = nc.vector, nc.scalar
            # alternate DVE/ACT (DVE 8/16ths + slight bias via pattern)
            yo_copy_eng = (V, A, V, A, V, A, V, A, V, A, V, A, V, A, V, A)
            e_out = (nc.sync, nc.gpsimd) * 8
            e_out2 = (nc.gpsimd, nc.sync) * 8
            for v in range(NVC):
                yo = yout_pool.tile([128, NCOL], f16, tag="y", name=f"yo{v}")
                for s in range(NSL):
                    sli = slice(s * SLW, (s + 1) * SLW)
                    ps = opsum.tile([128, SLW], f32, tag="o")
                    nc.tensor.matmul(
                        ps[:], wt_t[:, v * 128:(v + 1) * 128], xt[:, sli],
                        start=True, stop=False,
                    )
                    nc.tensor.matmul(
                        ps[:], g_t[:, v * 128:(v + 1) * 128], sv_all[:, sli],
                        start=False, stop=True,
                    )
                    eng = yo_copy_eng[(v * NSL + s) % 16]
                    if eng is nc.scalar:
                        eng.copy(yo[:, sli], ps[:])
                    else:
                        eng.tensor_copy(yo[:, sli], ps[:])
                e_out[v].dma_start(out_d[v, :, 0:768], yo[:, 0:768])
                e_out2[v].dma_start(out_d[v, :, 768:1536], yo[:, 768:1536])

    nc.compile()
    return nc


def _get_program():
    if "nc" not in _PROGRAM_CACHE:
        _PROGRAM_CACHE["nc"] = build_nc()
    return _PROGRAM_CACHE["nc"]


# --------------------------------------------------------------------------
# host driver
# --------------------------------------------------------------------------

def make_in_maps(x, a_coeffs, b_coeffs):
    x = np.asarray(x, np.float32)
    a = np.asarray(a_coeffs, np.float64)
    b = np.asarray(b_coeffs, np.float64)
    xf = x[:, 0, :]

    def to_rhs(x2d):
        xpad = np.zeros((B, TPAD), np.float32)
        xpad[:, :T] = x2d
        return np.ascontiguousarray(
            xpad.reshape(B, NBLK, L).transpose(2, 0, 1).reshape(128, NCOL)
        ).astype(np.float16)

    def to_rhs2(x2d):
        # chunk-interleaved block-major: X2[k, i*24 + c*8 + b]
        #   = xpad[b, (c*64+i)*128 + k]
        xpad = np.zeros((B, TPAD), np.float32)
        xpad[:, :T] = x2d
        return np.ascontiguousarray(
            xpad.reshape(B, 3, 64, L).transpose(3, 2, 1, 0).reshape(128, NCOL)
        ).astype(np.float16)

    Xf = to_rhs(xf)
    Xb = to_rhs(xf[:, ::-1])
    X2f = to_rhs2(xf)
    X2b = to_rhs2(xf[:, ::-1])
    ident = np.eye(128, dtype=np.float16)

    in_maps = []
    for core in range(8):
        fwd = core < 4
        chans = list(range((core % 4) * NVC, (core % 4) * NVC + NVC))
        tabs = _tables_for_channels(a, b, chans)
        in_maps.append(
            {
                "xrhs": Xf if fwd else Xb,
                "xrhs2": X2f if fwd else X2b,
                "ident": ident,
                **tabs,
            }
        )
    return in_maps


def assemble_output(core_outs):
    y = np.zeros((B, 2 * C, T), np.float32)
    for core in range(8):
        o = np.asarray(core_outs[core]).astype(np.float32) * BETA  # [16, 128, 1536]
        o = o.reshape(NVC, 128, B, NBLK).transpose(2, 0, 3, 1).reshape(B, NVC, TPAD)
        if core < 4:
            y[:, core * NVC:(core + 1) * NVC, :] = o[:, :, :T]
        else:
            y[:, C + (core - 4) * NVC:C + (core - 3) * NVC, :] = o[:, :, :T][:, :, ::-1]
    return y


def kernel(x, a_coeffs, b_coeffs, _trace=False):
    from concourse.bass_utils import run_bass_kernel_spmd

    nc = _get_program()
    in_maps = make_in_maps(x, a_coeffs, b_coeffs)
    res = run_bass_kernel_spmd(
        nc, in_maps, core_ids=list(range(8)), trace=_trace
    )
    y = assemble_output([r["out"] for r in res.results])
    if _trace:
        kernel.last_results = res
    return y


# revision 18
# speedup vs baseline: 1.5499x; 1.0528x over previous
"""Bidirectional 2nd-order IIR filter bank (64 channels) on 8 TRN2 NeuronCores.

Algorithm: block-parallel scan over the time axis (same math as the f32r
baseline, restructured for the v1 cost model where a DMA's engine charge is
free-dim bytes x 0.3855ns, min 500ns, on the dispatching engine).
  - T=24000 padded to 24576 = 192 blocks of L=128.
  - Phase A (per channel): zero-state particular solution of every block via a
    lower-triangular-Toeplitz matmul of the impulse response (PE, fp16).
  - Block states in REAL MODAL coordinates z = Vinv @ (y[n], y[n-1]); per-block
    increments p~ from a folded [128,2] matmul (xrhs2 layout).
  - Phase B: second-level scan as lower-block-triangular matmuls, 3 chunks of
    64 blocks; chunk2 adds a K=2 incoming-state matmul (sl).
  - States for ALL 16 filters are transposed per-batch (8 strided PE
    transposes) into ONE [32, NCOL] fp16 sv tile; phase C is then a K=32
    matmul with a host-built block-sparse g_all (rows 2v,2v+1 hold filter v's
    factors) accumulated onto phase A's PSUM. This avoids 16 expensive
    [2, NCOL] per-filter scatter DMAs entirely.
  - Everything on device is fp16 (PSUM accumulation f32) with global
    power-of-2 scaling: wt/beta, wp*alpha, g/(alpha*beta); the PSUM->SBUF
    output copy multiplies by beta. alpha=2^6, beta=2^-13 keeps every fp16
    stage in [1e-4, 200] (validated: rel_l2 ~5.4e-4 vs f64 reference).
Sharding: 128 (direction x channel) independent filters; cores 0-3 forward
channels 0-63, cores 4-7 backward channels 0-63, 16 filters/core, B=8 local.
Output DRAM layout is [vc, n, col] (col = b*192 + block) fp16; the final
transpose to [b, c, t] and f32 cast happen on host in numpy.

Perf notes (v1 cost model): DMA charge = free-bytes x 0.3855ns (x2 if the
min contiguous run < 512B), min 500ns, on the dispatching engine; DMA init
latency ~1.7us does not occupy the engine. Compute-op charge = free-size x
cycle_t + access bubble. Matmul charge = out free-size x 0.4167ns (fp16),
independent of K and partition count -- so thin-K matmuls and strided-AP
transposes are nearly free, and all small DMAs are batched (one sl DMA for
all 16 filters via the (r2, v, b) ss0 column layout).
"""

import sys

import numpy as np

if "/opt/trn_rl_repo" not in sys.path:
    sys.path.insert(0, "/opt/trn_rl_repo")

T = 24000
B = 8
C = 64
L = 128
NBLK = 192
TPAD = NBLK * L  # 24576
NCOL = B * NBLK  # 1536
NVC = 16  # filters per core
NSL = 3  # 512-wide column slices
SLW = 512

ALPHA = 2.0 ** 6
BETA = 2.0 ** -13

_PROGRAM_CACHE = {}


# --------------------------------------------------------------------------
# host-side table construction (float64 -> float16, device SBUF layouts)
# --------------------------------------------------------------------------

def _tables_for_channels(a, b, chans):
    """Per-core weight tables for 16 channels. a,b: [64,3] float64."""
    NV = len(chans)
    wt = np.zeros((NV, 128, 128), np.float64)
    wb = np.zeros((NV, 2, 2, 128, 128), np.float64)  # [v, r2, half, (c,i), j]
    wb2 = np.zeros((NV, 2, 2, 128, 64), np.float64)  # [v, r2, sc, (c,i), j']
    wp = np.zeros((128, 2 * NV), np.float64)
    ga = np.zeros((32, NV * 128), np.float64)  # rows 2v+r2, col block v

    for vi, ch in enumerate(chans):
        a1, a2, b0 = a[ch, 1], a[ch, 2], b[ch, 0]
        r = np.sqrt(a2)
        costh = -a1 / (2.0 * r)
        sinth = np.sqrt(max(0.0, 1.0 - costh * costh))
        th = np.arctan2(sinth, costh)

        # impulse response h[m] = b0 * phi(m), phi: homogeneous w/ phi(0)=1
        h = np.zeros(130)
        h[0] = b0
        h[1] = -a1 * b0
        for m in range(2, 130):
            h[m] = -a1 * h[m - 1] - a2 * h[m - 2]

        # Toeplitz lhsT: wt[k, m] = h[m-k] for m >= k
        k_i = np.arange(128)
        d = k_i[None, :] - k_i[:, None]  # [k, m] -> m - k
        wt[vi] = np.where(d >= 0, h[np.clip(d, 0, 129)], 0.0)

        # modal decomposition: A = V S Vinv,
        # V = [[r c, r s],[1,0]], S = r[[c, s],[-s, c]],
        # Vinv = [[0,1],[1/(r s), -c/s]]
        Vinv = np.array([[0.0, 1.0], [1.0 / (r * sinth), -costh / sinth]])

        # wp = W_T[:, [127,126]] @ Vinv.T  (p~ = wp.T @ x_block)
        wp[:, 2 * vi:2 * vi + 2] = wt[vi][:, [127, 126]] @ Vinv.T

        # g[c, n] = r^{n+2} (cos((n+2)th), sin((n+2)th)); block-sparse rows
        ks = np.arange(1, 129)
        rk = r ** (ks + 1.0)
        ga[2 * vi, vi * 128:(vi + 1) * 128] = rk * np.cos((ks + 1) * th)
        ga[2 * vi + 1, vi * 128:(vi + 1) * 128] = rk * np.sin((ks + 1) * th)

        # Mpow[q] = S^(128 q): scaled rotations
        qs = np.arange(0, 193)
        rq = r ** (128.0 * qs)
        ang = 128.0 * qs * th
        Mg = np.zeros((2, 2, 193))
        Mg[0, 0] = rq * np.cos(ang)
        Mg[0, 1] = rq * np.sin(ang)
        Mg[1, 0] = -Mg[0, 1]
        Mg[1, 1] = Mg[0, 0]

        # K-dim packing is c-major: kk = c*64 + i (i = block within chunk)
        ii = np.arange(64)
        jj = np.arange(128)
        j1 = np.arange(64)
        for r2 in range(2):
            for half in range(2):
                dd = jj[None, :] - (64 * half + ii[:, None])
                msk = dd >= 0
                dc = np.clip(dd, 0, 128)
                wb[vi, r2, half, 0:64, :] = np.where(msk, Mg[r2, 0, dc], 0.0)
                wb[vi, r2, half, 64:128, :] = np.where(msk, Mg[r2, 1, dc], 0.0)
            # chunk-2 states directly from p~ of chunks 0/1 (no sl round-trip):
            # power d = (128 + j') - (64*sc + i), j' in 0..63
            for sc in range(2):
                dd2 = 128 + j1[None, :] - 64 * sc - ii[:, None]
                wb2[vi, r2, sc, 0:64, :] = Mg[r2, 0, dd2]
                wb2[vi, r2, sc, 64:128, :] = Mg[r2, 1, dd2]

    # device SBUF layouts, fp16 with global scaling
    return {
        "wt": np.ascontiguousarray(
            (wt / BETA).transpose(1, 0, 2).reshape(128, NV * 128)
        ).astype(np.float16),
        "wb": np.ascontiguousarray(
            wb.transpose(3, 0, 1, 2, 4).reshape(128, NV * 2 * 2 * 128)
        ).astype(np.float16),
        "wb2": np.ascontiguousarray(
            wb2.transpose(3, 0, 1, 2, 4).reshape(128, NV * 2 * 2 * 64)
        ).astype(np.float16),
        "wp": (wp * ALPHA).astype(np.float16),
        "gall": (ga / (ALPHA * BETA)).astype(np.float16),
    }


# --------------------------------------------------------------------------
# device program
# --------------------------------------------------------------------------

def build_nc():
    """Build + compile the single-core Tile program (same on all 8 cores)."""
    import concourse.bass as bass
    import concourse.tile as tile
    from concourse import bacc, mybir

    f32 = mybir.dt.float32
    f16 = mybir.dt.float16

    nc = bacc.Bacc("TRN2", target_bir_lowering=False, debug=False)

    xr_d = nc.dram_tensor("xrhs", [128, NCOL], f16, kind="ExternalInput")
    xr2_d = nc.dram_tensor("xrhs2", [128, NCOL], f16, kind="ExternalInput")
    wt_d = nc.dram_tensor("wt", [128, NVC * 128], f16, kind="ExternalInput")
    wb_d = nc.dram_tensor("wb", [128, NVC * 512], f16, kind="ExternalInput")
    wb2_d = nc.dram_tensor("wb2", [128, NVC * 256], f16, kind="ExternalInput")
    wp_d = nc.dram_tensor("wp", [128, 2 * NVC], f16, kind="ExternalInput")
    ga_d = nc.dram_tensor("gall", [32, NVC * 128], f16, kind="ExternalInput")
    id_d = nc.dram_tensor("ident", [128, 128], f16, kind="ExternalInput")
    out_d = nc.dram_tensor("out", [NVC, 128, NCOL], f16, kind="ExternalOutput")

    with tile.TileContext(nc) as tc:
        with (
            tc.tile_pool(name="const", bufs=1) as const,
            tc.tile_pool(name="work", bufs=1) as work,
            tc.tile_pool(name="rp", bufs=16) as rp_pool,
            tc.tile_pool(name="yout", bufs=4) as yout_pool,
            tc.tile_pool(name="bpsum", bufs=4, space="PSUM") as bpsum,
            tc.tile_pool(name="opsum", bufs=3, space="PSUM") as opsum,
        ):
            # ---- constants into SBUF. Startup-latency aware: xrhs2/wp first
            # (feed p~), then the phase-B tables, then phase-A tables.
            xt2 = const.tile([128, NCOL], f16)
            nc.sync.dma_start(xt2[:, 0:768], xr2_d[:, 0:768])
            nc.scalar.dma_start(xt2[:, 768:1536], xr2_d[:, 768:1536])
            wp_t = const.tile([128, 2 * NVC], f16)
            nc.sync.dma_start(wp_t[:], wp_d[:])
            # phase-B tables, split per vc-pair/quad so early vcs unblock fast
            wb_t = const.tile([128, NVC * 512], f16)
            wb2_t = const.tile([128, NVC * 256], f16)
            wb_eng = (nc.gpsimd, nc.gpsimd, nc.scalar, nc.gpsimd)
            for q in range(8):
                wb_eng[q % 4].dma_start(
                    wb_t[:, q * 1024:(q + 1) * 1024], wb_d[:, q * 1024:(q + 1) * 1024]
                )
            for q in range(4):
                wb_eng[(q + 2) % 4].dma_start(
                    wb2_t[:, q * 1024:(q + 1) * 1024],
                    wb2_d[:, q * 1024:(q + 1) * 1024],
                )
            id_t = const.tile([128, 128], f16)
            nc.scalar.dma_start(id_t[:], id_d[:])

            # ---- persistent work tiles
            pp_all = work.tile([32, NCOL], f16)
            ss0_all = work.tile([128, 256], f16)  # cols b*32 + v*2 + r2
            ss1_all = work.tile([64, 256], f16)
            sv_all = work.tile([32, NCOL], f16)  # rows 2v+r2, cols b*192+n
            # zero once, early: covers the per-batch zero-state columns AND
            # the padding rows (16:32, 48:64) the K=64 phase-C rhs reads
            nc.vector.memset(sv_all[:], 0.0)

            # ---- p~ for all 16 vcs: one matmul per 512-col slice (xrhs2)
            pp_copy_eng = (nc.vector, nc.scalar, nc.vector)
            for s in range(3):
                pt = opsum.tile([32, SLW], f32, tag="o")
                nc.tensor.matmul(
                    pt[:], wp_t[:], xt2[:, s * SLW:(s + 1) * SLW],
                    start=True, stop=True,
                )
                eng = pp_copy_eng[s]
                if eng is nc.scalar:
                    eng.copy(pp_all[:, s * SLW:(s + 1) * SLW], pt[:])
                else:
                    eng.tensor_copy(pp_all[:, s * SLW:(s + 1) * SLW], pt[:])

            # ---- phase B: rp loads + s0 matmuls + ss0 copies (all vcs)
            rp_eng = (nc.sync, nc.scalar, nc.gpsimd) * 6
            s0_copy_eng = (nc.vector, nc.scalar, nc.vector, nc.scalar,
                           nc.vector, nc.vector, nc.scalar, nc.vector)
            rps = []
            for v in range(NVC):
                rp = rp_pool.tile([128, 24], f16, tag="rp", name=f"rp{v}")
                rp_eng[v].dma_start(rp[:], pp_all[2 * v:2 * v + 2, :])
                rps.append(rp)

            for v in range(NVC):
                rp = rps[v]
                s0 = bpsum.tile([128, 16], f32, tag="bp")
                for r2 in (0, 1):
                    base = v * 512 + r2 * 256
                    cs = slice(r2 * 8, r2 * 8 + 8)
                    nc.tensor.matmul(
                        s0[:, cs], wb_t[:, base:base + 128],
                        rp[:, 0:8], start=True, stop=False,
                    )
                    nc.tensor.matmul(
                        s0[:, cs], wb_t[:, base + 128:base + 256],
                        rp[:, 8:16], start=False, stop=True,
                    )
                # ss0_all[:, r2*128 + v*8 + b] <- s0[:, r2*8 + b]
                dst = ss0_all[:].rearrange("p (b w) -> p b w", b=8, w=32)
                src = s0[:].rearrange("p (r b) -> p b r", r=2, b=8)
                eng = s0_copy_eng[v % 8]
                if eng is nc.scalar:
                    eng.copy(dst[:, :, 2 * v:2 * v + 2], src)
                else:
                    eng.tensor_copy(dst[:, :, 2 * v:2 * v + 2], src)

            # ---- s1 matmuls + ss1 copies (chunk-2 states straight from p~)
            for v in range(NVC):
                rp = rps[v]
                s1 = bpsum.tile([64, 16], f32, tag="bp")
                for r2 in (0, 1):
                    base = v * 512 + r2 * 256
                    b2 = v * 256 + r2 * 128
                    cs = slice(r2 * 8, r2 * 8 + 8)
                    nc.tensor.matmul(
                        s1[:, cs], wb2_t[:, b2:b2 + 64],
                        rp[:, 0:8], start=True, stop=False,
                    )
                    nc.tensor.matmul(
                        s1[:, cs], wb2_t[:, b2 + 64:b2 + 128],
                        rp[:, 8:16], start=False, stop=False,
                    )
                    nc.tensor.matmul(
                        s1[:, cs], wb_t[:, base:base + 64],
                        rp[:, 16:24], start=False, stop=True,
                    )
                dst = ss1_all[:].rearrange("p (b w) -> p b w", b=8, w=32)
                src = s1[:].rearrange("p (r b) -> p b r", r=2, b=8)
                eng = s0_copy_eng[v % 8]
                if eng is nc.scalar:
                    eng.copy(dst[:, :, 2 * v:2 * v + 2], src)
                else:
                    eng.tensor_copy(dst[:, :, 2 * v:2 * v + 2], src)

            # ---- phase-A tables (needed only once sv_all is ready)
            xt = const.tile([128, NCOL], f16)
            nc.sync.dma_start(xt[:], xr_d[:])
            wt_t = const.tile([128, NVC * 128], f16)
            nc.sync.dma_start(wt_t[:], wt_d[:])
            g_t = const.tile([32, NVC * 128], f16)
            nc.gpsimd.dma_start(g_t[:], ga_d[:])

            # ---- per-batch transposes: all 16 vcs x 2 comps in one shot,
            # inputs are plain contiguous [*, 32] slices
            psva = [bpsum.tile([32, 512], f16, tag="bp", name=f"psva{h}")
                    for h in range(2)]
            psvb = bpsum.tile([32, 512], f16, tag="bp", name="psvb")
            for b in range(8):
                nc.tensor.transpose(
                    psva[b // 4][:, (b % 4) * 128:(b % 4 + 1) * 128],
                    ss0_all[:, b * 32:(b + 1) * 32], id_t[:],
                )
                nc.tensor.transpose(
                    psvb[:, b * 64:(b + 1) * 64],
                    ss1_all[:, b * 32:(b + 1) * 32], id_t[0:64, 0:64],
                )
            # sv_all[2v+r2, b*192+n] = s_{n-1}; col b*192 stays zero
            dst = sv_all[:].rearrange("p (b n) -> p b n", b=8, n=192)
            for h in range(2):
                ina = psva[h][:].rearrange("p (b n) -> p b n", b=4, n=128)
                nc.vector.tensor_copy(
                    dst[:, 4 * h:4 * h + 4, 1:129], ina
                )
            inb = psvb[:].rearrange("p (b n) -> p b n", b=8, n=64)[:, :, 0:63]
            nc.scalar.copy(dst[:, :, 129:192], inb)

            # ---- phases A + C per 512-col slice; K=32 block-sparse g matmul
            # accumulates the state correction onto the phase-A PSUM.
            V, A = nc.vector, nc.scalar
            # alternate DVE/ACT (DVE 8/16ths + slight bias via pattern)
            yo_copy_eng = (V, A, V, A, V, A, V, A, V, A, V, A, V, A, V, A)
            e_out = (nc.sync, nc.gpsimd) * 8
            e_out2 = (nc.gpsimd, nc.sync) * 8
            for v in range(NVC):
                yo = yout_pool.tile([128, NCOL], f16, tag="y", name=f"yo{v}")
                for s in range(NSL):
                    sli = slice(s * SLW, (s + 1) * SLW)
                    ps = opsum.tile([128, SLW], f32, tag="o")
                    nc.tensor.matmul(
                        ps[:], wt_t[:, v * 128:(v + 1) * 128], xt[:, sli],
                        start=True, stop=False,
                    )
                    nc.tensor.matmul(
                        ps[:], g_t[:, v * 128:(v + 1) * 128], sv_all[:, sli],
                        start=False, stop=True,
                    )
                    eng = yo_copy_eng[(v * NSL + s) % 16]
                    if eng is nc.scalar:
                        eng.copy(yo[:, sli], ps[:])
                    else:
                        eng.tensor_copy(yo[:, sli], ps[:])
                e_out[v].dma_start(out_d[v, :, 0:768], yo[:, 0:768])
                e_out2[v].dma_start(out_d[v, :, 768:1536], yo[:, 768:1536])

    nc.compile()
    return nc


def _get_program():
    if "nc" not in _PROGRAM_CACHE:
        _PROGRAM_CACHE["nc"] = build_nc()
    return _PROGRAM_CACHE["nc"]


# --------------------------------------------------------------------------
# host driver
# --------------------------------------------------------------------------

def make_in_maps(x, a_coeffs, b_coeffs):
    x = np.asarray(x, np.float32)
    a = np.asarray(a_coeffs, np.float64)
    b = np.asarray(b_coeffs, np.float64)
    xf = x[:, 0, :]

    def to_rhs(x2d):
        xpad = np.zeros((B, TPAD), np.float32)
        xpad[:, :T] = x2d
        return np.ascontiguousarray(
            xpad.reshape(B, NBLK, L).transpose(2, 0, 1).reshape(128, NCOL)
        ).astype(np.float16)

    def to_rhs2(x2d):
        # chunk-interleaved block-major: X2[k, i*24 + c*8 + b]
        #   = xpad[b, (c*64+i)*128 + k]
        xpad = np.zeros((B, TPAD), np.float32)
        xpad[:, :T] = x2d
        return np.ascontiguousarray(
            xpad.reshape(B, 3, 64, L).transpose(3, 2, 1, 0).reshape(128, NCOL)
        ).astype(np.float16)

    Xf = to_rhs(xf)
    Xb = to_rhs(xf[:, ::-1])
    X2f = to_rhs2(xf)
    X2b = to_rhs2(xf[:, ::-1])
    ident = np.eye(128, dtype=np.float16)

    in_maps = []
    for core in range(8):
        fwd = core < 4
        chans = list(range((core % 4) * NVC, (core % 4) * NVC + NVC))
        tabs = _tables_for_channels(a, b, chans)
        in_maps.append(
            {
                "xrhs": Xf if fwd else Xb,
                "xrhs2": X2f if fwd else X2b,
                "ident": ident,
                **tabs,
            }
        )
    return in_maps


def assemble_output(core_outs):
    y = np.zeros((B, 2 * C, T), np.float32)
    for core in range(8):
        o = np.asarray(core_outs[core]).astype(np.float32) * BETA  # [16, 128, 1536]
        o = o.reshape(NVC, 128, B, NBLK).transpose(2, 0, 3, 1).reshape(B, NVC, TPAD)
        if core < 4:
            y[:, core * NVC:(core + 1) * NVC, :] = o[:, :, :T]
        else:
            y[:, C + (core - 4) * NVC:C + (core - 3) * NVC, :] = o[:, :, :T][:, :, ::-1]
    return y


def kernel(x, a_coeffs, b_coeffs, _trace=False):
    from concourse.bass_utils import run_bass_kernel_spmd

    nc = _get_program()
    in_maps = make_in_maps(x, a_coeffs, b_coeffs)
    res = run_bass_kernel_spmd(
        nc, in_maps, core_ids=list(range(8)), trace=_trace
    )
    y = assemble_output([r["out"] for r in res.results])
    if _trace:
        kernel.last_results = res
    return y
